# revision 15
# baseline (speedup 1.0000x reference)
"""Trainium2 Bass kernel for nn_ODESurvMultiple (dense_mlp, 8-core data parallel).

reference math (per sample row x[256], scalar t):
  pi    = softmax(relu(x@W1p+b1p) @ W2p + b2p)                      [K=8]
  g     = x @ W1o[:-1] + b1o                                        [H=512]
  h_n   = relu(g + c_n * (t * w))     c_n=(1+u_n)/2, w=W1o[-1]      [NQ, 512]
  f_n   = softplus(h_n @ W2o + b2o)                                 [NQ, 8]
  pred  = (t/2) * sum_n W_n f_n                                     [8]
  preds = pi * (1 - exp(-pred))
returns (preds, pi)

Implementation notes:
- NQ quadrature nodes (reference uses 15; Gauss-Legendre converges so fast on
  this integrand that NQ=6 matches the 15-node reference to ~7e-5, far inside
  the 2e-2 gate; bf16 rounding dominates the error at ~4e-3).
- bf16 operands everywhere on the PE; fp32 PSUM accumulation.
- layer-2 packs up to 4 quadrature nodes (and the pi-head logits) into one
  PSUM tile via column tile_position strips -> 4 concurrent matmuls, and the
  softplus + quadrature sum then run at 104-partition width instead of 8.
- softplus is a single ACT pass (softplus_and_others table also has relu).
- quadrature sum is a PE matmul against a strip-weight matrix R with
  -W_n/2 folded in; the (t) factor is applied during the psum evacuation.
- h build: mix of DVE route (stt: t_bcast*w_pc + g, then relu) and PE route
  (identity re-inject + rank-1 into psum, relu-evac on ACT/DVE).
"""

import os
import sys

for _p in (
    "/root/.axon_site",
    "/root/.axon_site/_ro/trn_rl_repo",
    "/root/.axon_site/_ro/pypackages",
    "/opt/trn_rl_repo",
):
    if os.path.isdir(_p) and _p not in sys.path:
        sys.path.append(_p)

import numpy as np

import concourse.bass as bass
import concourse.mybir as mybir
import concourse.tile as tile
from concourse import bacc
from concourse.bass_utils import run_bass_kernel_spmd
from concourse.masks import make_identity

F32 = mybir.dt.float32
BF = mybir.dt.bfloat16
AX = mybir.AxisListType
OP = mybir.AluOpType
AF = mybir.ActivationFunctionType

# Steer the greedy act-table selector: keep set ORDER identical (the emitted
# act_func_set_id is a positional index), but hide Exp/Ln/Relu/Copy/Identity
# from all other sets so the whole kernel uses the one combined set (1 load).
_orig_get_tables = bacc.get_activation_tables


def _tables_lnexp_first(arch):
    t = _orig_get_tables(arch)
    pref = "natural_log_exp_and_others"
    if pref not in t:
        return t
    hide = {AF.Exp, AF.Ln, AF.Relu, AF.Copy, AF.Identity}
    out = {}
    for k, v in t.items():
        if k != pref and (v & hide):
            v = v - hide
        out[k] = v
    return out


bacc.get_activation_tables = _tables_lnexp_first

N_CORES = 8
B_FULL, COV, H, K = 16384, 256, 512, 8
B = B_FULL // N_CORES  # 2048 per core
TT, TS = 4, 512        # batch column tiles
C = H // 128           # 4 H-chunks
CIN = COV // 128       # 2 cov-chunks

# --- tuning knobs -----------------------------------------------------------
NQ = 6                 # quadrature nodes (ref=15; 6 -> 7e-5 quad error)
PE_ROUTE = 10          # of NQ*C h units, how many go PE+evac (rest DVE stt)
PE_EVAC_DVE = 0        # of the PE-route units, how many evac on DVE (rest ACT)
RELU_ACT = 0           # of the DVE-route units, how many relu on ACT
RELU_GPS = 4           # of the DVE-route units, how many relu on GPSIMD
G_EVAC_ACT = 8         # of the 16 g evacs, how many on ACT (rest DVE)
H1P_EVAC_DVE = 0       # of the 16 h1p evacs, how many on DVE (rest ACT)
# ---------------------------------------------------------------------------

_u64, _w64 = np.polynomial.legendre.leggauss(NQ)
CN = [float(np.float32(0.5) * (np.float32(1.0) + u)) for u in _u64.astype(np.float32)]
WN = [float(w) for w in _w64.astype(np.float32)]

# layer-2 strip groups: chunks of up to 4 nodes; pi head rides in the last
# group's strip 3 (or its own group if the last one is full).
_node_groups = [list(range(i, min(i + 4, NQ))) for i in range(0, NQ, 4)]
if len(_node_groups[-1]) <= 3:
    PI_GROUP = len(_node_groups) - 1
else:
    _node_groups.append([])
    PI_GROUP = len(_node_groups) - 1
PI_STRIP = 3
NGROUPS = len(_node_groups)


def _spread(n_total, count):
    return {i for i in range(n_total) if ((i + 1) * count) // n_total > (i * count) // n_total}


def build_kernel():
    nc = bacc.Bacc("TRN2", target_bir_lowering=False, debug=False)

    x_d = nc.dram_tensor("x", [B, COV], F32, kind="ExternalInput").ap()
    t_d = nc.dram_tensor("t", [B], F32, kind="ExternalInput").ap()
    w1p_d = nc.dram_tensor("W1p", [COV, H], F32, kind="ExternalInput").ap()
    b1p_d = nc.dram_tensor("b1p", [H], F32, kind="ExternalInput").ap()
    w2p_d = nc.dram_tensor("W2p", [H, K], F32, kind="ExternalInput").ap()
    b2p_d = nc.dram_tensor("b2p", [K], F32, kind="ExternalInput").ap()
    w1o_d = nc.dram_tensor("W1o", [COV + 1, H], F32, kind="ExternalInput").ap()
    b1o_d = nc.dram_tensor("b1o", [H], F32, kind="ExternalInput").ap()
    w2o_d = nc.dram_tensor("W2o", [H, K], F32, kind="ExternalInput").ap()
    b2o_d = nc.dram_tensor("b2o", [K], F32, kind="ExternalInput").ap()
    preds_d = nc.dram_tensor("preds", [B, K], F32, kind="ExternalOutput").ap()
    pi_d = nc.dram_tensor("pi", [B, K], F32, kind="ExternalOutput").ap()

    n_units = NQ * C
    pe_units = _spread(n_units, PE_ROUTE)
    pe_units_l = sorted(pe_units)
    pe_evac_dve = {pe_units_l[i] for i in sorted(_spread(len(pe_units_l), PE_EVAC_DVE))} if pe_units_l else set()
    dve_units_l = sorted(set(range(n_units)) - pe_units)
    relu_act = {dve_units_l[i] for i in sorted(_spread(len(dve_units_l), RELU_ACT))} if dve_units_l else set()
    rest_l = [i for i in dve_units_l if i not in relu_act]
    relu_gps = {rest_l[i] for i in sorted(_spread(len(rest_l), RELU_GPS))} if rest_l else set()
    g_evac_act = _spread(16, G_EVAC_ACT)
    h1p_evac_dve = _spread(16, H1P_EVAC_DVE)

    with tile.TileContext(nc) as tc:
        with (
            tc.tile_pool(name="pers", bufs=1) as pers,
            tc.tile_pool(name="ph", bufs=n_units) as ph,
            tc.tile_pool(name="pxin", bufs=3) as pxin,
            tc.tile_pool(name="pft", bufs=4) as pft,
            tc.tile_pool(name="psm", bufs=1) as psm,
            tc.tile_pool(name="pps", bufs=4, space="PSUM") as pps,
            tc.tile_pool(name="ppsf", bufs=2, space="PSUM") as ppsf,
            tc.tile_pool(name="ppred", bufs=2, space="PSUM") as ppred,
        ):
            def pt(name, shape, dt=F32):
                return pers.tile(shape, dt, tag=name, name=name)

            # ---- persistent SBUF tiles ----
            ident128 = pt("ident128", [128, 128])          # fp32, for x transpose
            identB = pt("identB", [128, 128], BF)          # bf16, for PE h route
            ident8 = pt("ident8", [8, 8])                  # fp32, small transposes
            xT = pt("xT", [128, CIN * B], BF)              # [128, ci*2048+b]
            g_sb = [pt(f"g{c}", [128, B], BF) for c in range(C)]
            h1p_sb = [pt(f"h1p{c}", [128, B], BF) for c in range(C)]
            t_bcast = pt("t_bcast", [128, B], BF)
            t_row_bf = pt("t_row_bf", [1, B], BF)
            ones_row = pt("ones_row", [1, 128], BF)
            w1o_sb = [pt(f"w1o{ci}", [128, H], BF) for ci in range(CIN)]
            w1p_sb = [pt(f"w1p{ci}", [128, H], BF) for ci in range(CIN)]
            w2o_sb = [pt(f"w2o{c}", [128, K], BF) for c in range(C)]
            w2p_sb = [pt(f"w2p{c}", [128, K], BF) for c in range(C)]
            w_row = pt("w_row", [1, H])                    # fp32 W1o[-1]
            w_pc = pt("w_pc", [128, C])                    # fp32 W1o[-1] as [p,c]
            wsc_row = [pt(f"wscr{n}", [1, H], BF) for n in range(NQ)]
            wsc_pc = [pt(f"wscp{n}", [128, C], BF) for n in range(NQ)]
            b1o_pc = pt("b1o_pc", [128, C])
            b1p_pc = pt("b1p_pc", [128, C])
            b2o_col = pt("b2o_col", [128, 1])
            b2p_col = pt("b2p_col", [128, 1])
            R_sb = [pt(f"R{g}", [128, 8], BF) for g in range(NGROUPS)]
            pred_sb = pt("pred_sb", [8, B])
            lgt_sb = pt("lgt_sb", [8, B])
            pred_b = pt("pred_b", [128, B // 128 * K])
            logits_b = pt("logits_b", [128, B // 128 * K])
            e_b = pt("e_b", [128, B // 128 * K])
            eneg = pt("eneg", [128, B // 128 * K])
            sums = pt("sums", [128, B // 128])
            rec = pt("rec", [128, B // 128])
            pi_b = pt("pi_b", [128, B // 128 * K])
            cif_b = pt("cif_b", [128, B // 128 * K])
            preds_b = pt("preds_b", [128, B // 128 * K])

            # ---- constants ----
            make_identity(nc, ident128)
            make_identity(nc, identB)
            make_identity(nc, ident8)
            nc.vector.memset(ones_row, 1.0)
            nc.vector.memset(b2o_col, 0.0)
            nc.vector.memset(b2p_col, 0.0)

            # ---- weight / small input DMAs + bf16 casts ----
            for ci in range(CIN):
                w1o_ld = psm.tile([128, H], F32, tag="wld", name=f"w1old{ci}")
                nc.sync.dma_start(out=w1o_ld, in_=w1o_d[ci * 128 : (ci + 1) * 128, :])
                nc.vector.tensor_copy(w1o_sb[ci], w1o_ld)
                w1p_ld = psm.tile([128, H], F32, tag="wld2", name=f"w1pld{ci}")
                nc.sync.dma_start(out=w1p_ld, in_=w1p_d[ci * 128 : (ci + 1) * 128, :])
                nc.vector.tensor_copy(w1p_sb[ci], w1p_ld)
            for c in range(C):
                w2o_ld = psm.tile([128, K], F32, tag="w2ld", name=f"w2old{c}")
                nc.sync.dma_start(out=w2o_ld, in_=w2o_d[c * 128 : (c + 1) * 128, :])
                nc.vector.tensor_copy(w2o_sb[c], w2o_ld)
                w2p_ld = psm.tile([128, K], F32, tag="w2ld2", name=f"w2pld{c}")
                nc.sync.dma_start(out=w2p_ld, in_=w2p_d[c * 128 : (c + 1) * 128, :])
                nc.vector.tensor_copy(w2p_sb[c], w2p_ld)

            t_row_ld = pers.tile([1, B], F32, tag="trow", name="t_row_ld")
            nc.sync.dma_start(out=t_row_ld, in_=t_d.rearrange("(a b) -> a b", a=1))
            nc.vector.tensor_copy(t_row_bf, t_row_ld)
            nc.sync.dma_start(out=w_row, in_=w1o_d[COV : COV + 1, :])
            nc.sync.dma_start(
                out=w_pc, in_=w1o_d[COV : COV + 1, :].rearrange("a (c p) -> p (c a)", p=128)
            )
            for n in range(NQ):
                nc.vector.tensor_scalar_mul(wsc_row[n], w_row, CN[n])
                nc.vector.tensor_scalar_mul(wsc_pc[n], w_pc, CN[n])
            nc.sync.dma_start(out=b1o_pc, in_=b1o_d.rearrange("(c p) -> p c", p=128))
            nc.sync.dma_start(out=b1p_pc, in_=b1p_d.rearrange("(c p) -> p c", p=128))
            for j in range(4):
                nc.sync.dma_start(
                    out=b2o_col[32 * j : 32 * j + 8, :],
                    in_=b2o_d.rearrange("(k a) -> k a", a=1),
                )
            nc.sync.dma_start(
                out=b2p_col[32 * PI_STRIP : 32 * PI_STRIP + 8, :],
                in_=b2p_d.rearrange("(k a) -> k a", a=1),
            )
            # strip-weight matrices: R[g][32j+k, k] = -0.5 * WN[node], else 0
            for g, nodes in enumerate(_node_groups):
                nc.vector.memset(R_sb[g], 0.0)
                for j, n in enumerate(nodes):
                    nc.scalar.activation(
                        R_sb[g][32 * j : 32 * j + 8, :], ident8, AF.Copy,
                        scale=-0.5 * WN[n],
                    )

            # ---- x load + transpose + cast to bf16 (feature-major xT) ----
            # 4 transposes (2 xin tiles x 2 cov chunks) batch into one psum
            # bank; one ACT copy evacuates them (cast to bf16) via a 4D AP.
            xT_v = xT.rearrange("p (ci b) -> p ci b", ci=CIN)
            for half in range(B // 256):
                pxt = pps.tile([128, 512], F32, tag="ps", name=f"pxt_{half}")
                for jj in range(2):
                    r = half * 256 + jj * 128
                    xin = pxin.tile([128, COV], F32, tag="xin", name=f"xin_{half}_{jj}")
                    # alternate the two HWDGE queues (sync + scalar) so the
                    # 2MB x load doesn't serialize on one queue
                    dma_eng = nc.sync if (half * 2 + jj) % 2 == 0 else nc.scalar
                    dma_eng.dma_start(out=xin, in_=x_d[r : r + 128, :])
                    for ci in range(CIN):
                        nc.tensor.transpose(
                            pxt[:, (jj * 2 + ci) * 128 : (jj * 2 + ci + 1) * 128],
                            xin[:, ci * 128 : (ci + 1) * 128],
                            ident128,
                        )
                nc.scalar.copy(
                    xT_v[:, :, half * 256 : (half + 1) * 256].rearrange(
                        "p ci (jj q) -> p jj ci q", jj=2
                    ),
                    pxt.rearrange("p (jj ci q) -> p jj ci q", jj=2, ci=CIN),
                )

            # ---- t_bcast[p, b] = t[b] (rank-1 ones x t) ----
            for T in range(TT):
                bs = slice(T * TS, (T + 1) * TS)
                pst = pps.tile([128, TS], F32, tag="ps", name=f"ptb_{T}")
                nc.tensor.matmul(pst, ones_row, t_row_bf[:, bs], start=True, stop=True)
                nc.vector.tensor_copy(t_bcast[:, bs], pst)

            # ---- layer-1 matmuls (both nets) ----
            for c in range(C):
                cs = slice(c * 128, (c + 1) * 128)
                for T in range(TT):
                    bs = slice(T * TS, (T + 1) * TS)
                    i = c * TT + T
                    pso = pps.tile([128, TS], F32, tag="ps", name=f"pso_{c}_{T}")
                    for ci in range(CIN):
                        nc.tensor.matmul(
                            pso, w1o_sb[ci][:, cs],
                            xT_v[:, ci, T * TS : (T + 1) * TS],
                            start=(ci == 0), stop=(ci == CIN - 1),
                        )
                    if i in g_evac_act:
                        nc.scalar.activation(
                            g_sb[c][:, bs], pso, AF.Identity, bias=b1o_pc[:, c : c + 1]
                        )
                    else:
                        nc.vector.tensor_scalar_add(
                            g_sb[c][:, bs], pso, b1o_pc[:, c : c + 1]
                        )
                    psp = pps.tile([128, TS], F32, tag="ps", name=f"psp_{c}_{T}")
                    for ci in range(CIN):
                        nc.tensor.matmul(
                            psp, w1p_sb[ci][:, cs],
                            xT_v[:, ci, T * TS : (T + 1) * TS],
                            start=(ci == 0), stop=(ci == CIN - 1),
                        )
                    if i in h1p_evac_dve:
                        nc.vector.tensor_scalar(
                            h1p_sb[c][:, bs], psp, b1p_pc[:, c : c + 1], 0.0,
                            OP.add, OP.max,
                        )
                    else:
                        nc.scalar.activation(
                            h1p_sb[c][:, bs], psp, AF.Relu, bias=b1p_pc[:, c : c + 1]
                        )

            # ---- h units ----
            h_tiles = {}
            for n in range(NQ):
                for c in range(C):
                    i = n * C + c
                    ht = ph.tile([128, B], BF, tag="h", name=f"h_{n}_{c}")
                    if i in pe_units:
                        cs = slice(c * 128, (c + 1) * 128)
                        # group the identity matmuls then the rank-1s so the
                        # stationary operand only reloads twice per unit
                        pshs = []
                        for T in range(TT):
                            bs = slice(T * TS, (T + 1) * TS)
                            psh = pps.tile([128, TS], F32, tag="ps", name=f"psh_{n}_{c}_{T}")
                            nc.tensor.matmul(psh, identB, g_sb[c][:, bs], start=True, stop=False)
                            pshs.append(psh)
                        for T in range(TT):
                            bs = slice(T * TS, (T + 1) * TS)
                            nc.tensor.matmul(
                                pshs[T], wsc_row[n][:, cs], t_row_bf[:, bs],
                                start=False, stop=True,
                            )
                        for T in range(TT):
                            bs = slice(T * TS, (T + 1) * TS)
                            if i in pe_evac_dve:
                                nc.vector.tensor_scalar_max(ht[:, bs], pshs[T], 0.0)
                            else:
                                nc.scalar.activation(ht[:, bs], pshs[T], AF.Relu)
                    else:
                        nc.vector.scalar_tensor_tensor(
                            out=ht, in0=t_bcast, scalar=wsc_pc[n][:, c : c + 1],
                            in1=g_sb[c], op0=OP.mult, op1=OP.add,
                        )
                        if i in relu_act:
                            nc.scalar.activation(ht, ht, AF.Relu)
                        elif i in relu_gps:
                            nc.gpsimd.tensor_scalar_max(ht, ht, 0.0)
                        else:
                            nc.vector.tensor_scalar_max(ht, ht, 0.0)
                    h_tiles[(n, c)] = ht

            # ---- layer-2 (col-tiled strips) + softplus + quadrature,
            #      T-major so the tail work of early T overlaps later T ----
            fgroups = [g for g, nodes in enumerate(_node_groups) if nodes]
            psf_count = 0
            for T in range(TT):
                bs = slice(T * TS, (T + 1) * TS)
                f_tiles = {}
                for g, nodes in enumerate(_node_groups):
                    psf = ppsf.tile([128, TS], F32, tag="psf", name=f"psf_{g}_{T}")
                    if psf_count < 2:
                        # first touch of each ring slot: clear garbage rows so
                        # exp of unwritten partitions stays finite
                        nc.vector.memset(psf, 0.0)
                    psf_count += 1
                    for c in range(C):
                        for j, n in enumerate(nodes):
                            nc.tensor.matmul(
                                psf[32 * j : 32 * j + 8, :],
                                w2o_sb[c], h_tiles[(n, c)][:, bs],
                                start=(c == 0), stop=(c == C - 1),
                                tile_position=(0, 32 * j),
                            )
                        if g == PI_GROUP:
                            nc.tensor.matmul(
                                psf[32 * PI_STRIP : 32 * PI_STRIP + 8, :],
                                w2p_sb[c], h1p_sb[c][:, bs],
                                start=(c == 0), stop=(c == C - 1),
                                tile_position=(0, 32 * PI_STRIP),
                            )
                    if nodes:
                        top = 32 * (len(nodes) - 1) + 8
                        et = pft.tile([top, TS], F32, tag="et", name=f"et_{g}_{T}")
                        nc.scalar.activation(
                            et, psf[0:top, :], AF.Exp, bias=b2o_col[0:top, :]
                        )
                        ft = pft.tile([top, TS], BF, tag="ft", name=f"ft_{g}_{T}")
                        nc.scalar.activation(ft, et, AF.Ln, bias=1.0)
                        f_tiles[g] = ft
                    if g == PI_GROUP:
                        nc.vector.tensor_scalar_add(
                            lgt_sb[:, bs],
                            psf[32 * PI_STRIP : 32 * PI_STRIP + 8, :],
                            b2p_col[32 * PI_STRIP : 32 * PI_STRIP + 8, :],
                        )

                # quadrature: pred_sb = -(t/2) sum_n W_n f_n
                pp = ppred.tile([8, TS], F32, tag="ppred", name=f"ppred_{T}")
                for k, g in enumerate(fgroups):
                    top = 32 * (len(_node_groups[g]) - 1) + 8
                    nc.tensor.matmul(
                        pp, R_sb[g][0:top, :], f_tiles[g],
                        start=(k == 0), stop=(k == len(fgroups) - 1),
                    )
                # fold in the per-column t factor during evacuation
                nc.vector.scalar_tensor_tensor(
                    out=pred_sb[:, bs], in0=pp, scalar=1.0,
                    in1=t_bcast[0:8, bs], op0=OP.mult, op1=OP.mult,
                )

                # transpose pred + logits to batch-major
                pxp = pps.tile([128, 32], F32, tag="ps", name=f"pxp_{T}")
                pxl = pps.tile([128, 32], F32, tag="ps", name=f"pxl_{T}")
                for j in range(4):
                    off = T * TS + j * 128
                    nc.tensor.transpose(
                        pxp[:, j * 8 : (j + 1) * 8], pred_sb[:, off : off + 128], ident8
                    )
                    nc.tensor.transpose(
                        pxl[:, j * 8 : (j + 1) * 8], lgt_sb[:, off : off + 128], ident8
                    )
                nc.vector.tensor_copy(pred_b[:, T * 32 : (T + 1) * 32], pxp)
                nc.vector.tensor_copy(logits_b[:, T * 32 : (T + 1) * 32], pxl)

            # ---- final: softmax(pi), cif, preds (table switch to exp here) ----
            nc.scalar.activation(e_b, logits_b, AF.Exp)
            nc.scalar.activation(eneg, pred_b, AF.Exp)
            nc.vector.tensor_reduce(
                sums, e_b.rearrange("p (t k) -> p t k", k=8), axis=AX.X, op=OP.add
            )
            nc.vector.reciprocal(rec, sums)
            for jj in range(B // 128):
                nc.vector.tensor_scalar_mul(
                    pi_b[:, jj * 8 : (jj + 1) * 8],
                    e_b[:, jj * 8 : (jj + 1) * 8],
                    rec[:, jj : jj + 1],
                )
            nc.vector.tensor_scalar(cif_b, eneg, -1.0, 1.0, OP.mult, OP.add)
            nc.vector.tensor_tensor(out=preds_b, in0=cif_b, in1=pi_b, op=OP.mult)

            nc.sync.dma_start(
                out=preds_d.rearrange("(j p) k -> p j k", p=128),
                in_=preds_b.rearrange("p (j k) -> p j k", k=8),
            )
            nc.sync.dma_start(
                out=pi_d.rearrange("(j p) k -> p j k", p=128),
                in_=pi_b.rearrange("p (j k) -> p j k", k=8),
            )

    nc.compile()
    return nc


_NC = None


def _get_nc():
    global _NC
    if _NC is None:
        _NC = build_kernel()
    return _NC


def _shard_inputs(inputs):
    in_maps = []
    for i in range(N_CORES):
        sl = slice(i * B, (i + 1) * B)
        m = {
            "x": np.ascontiguousarray(np.asarray(inputs["x"], np.float32)[sl]),
            "t": np.ascontiguousarray(np.asarray(inputs["t"], np.float32)[sl]),
        }
        for k in ("W1p", "b1p", "W2p", "b2p", "W1o", "b1o", "W2o", "b2o"):
            m[k] = np.asarray(inputs[k], np.float32)
        in_maps.append(m)
    return in_maps


def kernel(**inputs):
    nc = _get_nc()
    in_maps = _shard_inputs(inputs)
    res = run_bass_kernel_spmd(nc, in_maps, core_ids=list(range(N_CORES)))
    preds = np.concatenate([res.results[i]["preds"] for i in range(N_CORES)], axis=0)
    pi = np.concatenate([res.results[i]["pi"] for i in range(N_CORES)], axis=0)
    return (preds, pi)


# revision 16
# speedup vs baseline: 1.6496x; 1.6496x over previous
"""Trainium2 Bass kernel for nn_ODESurvMultiple (dense_mlp, 8-core data parallel).

reference math (per sample row x[256], scalar t):
  pi    = softmax(relu(x@W1p+b1p) @ W2p + b2p)                      [K=8]
  g     = x @ W1o[:-1] + b1o                                        [H=512]
  h_n   = relu(g + c_n * (t * w))     c_n=(1+u_n)/2, w=W1o[-1]      [NQ, 512]
  f_n   = softplus(h_n @ W2o + b2o)                                 [NQ, 8]
  pred  = (t/2) * sum_n W_n f_n                                     [8]
  preds = pi * (1 - exp(-pred))
returns (preds, pi)

Implementation notes:
- NQ quadrature nodes (reference uses 15; Gauss-Legendre converges so fast on
  this integrand that NQ=6 matches the 15-node reference to ~7e-5, far inside
  the 2e-2 gate; bf16 rounding dominates the error at ~4e-3).
- bf16 operands everywhere on the PE; fp32 PSUM accumulation.
- layer-2 packs up to 4 quadrature nodes (and the pi-head logits) into one
  PSUM tile via column tile_position strips -> 4 concurrent matmuls, and the
  softplus + quadrature sum then run at 104-partition width instead of 8.
- softplus is a single ACT pass (softplus_and_others table also has relu).
- quadrature sum is a PE matmul against a strip-weight matrix R with
  -W_n/2 folded in; the (t) factor is applied during the psum evacuation.
- h build: mix of DVE route (stt: t_bcast*w_pc + g, then relu) and PE route
  (identity re-inject + rank-1 into psum, relu-evac on ACT/DVE).
"""

import os
import sys

for _p in (
    "/root/.axon_site",
    "/root/.axon_site/_ro/trn_rl_repo",
    "/root/.axon_site/_ro/pypackages",
    "/opt/trn_rl_repo",
):
    if os.path.isdir(_p) and _p not in sys.path:
        sys.path.append(_p)

import numpy as np

import concourse.bass as bass
import concourse.mybir as mybir
import concourse.tile as tile
from concourse import bacc
from concourse.bass_utils import run_bass_kernel_spmd
from concourse.masks import make_identity

F32 = mybir.dt.float32
BF = mybir.dt.bfloat16
AX = mybir.AxisListType
OP = mybir.AluOpType
AF = mybir.ActivationFunctionType

# Steer the greedy act-table selector: keep set ORDER identical (the emitted
# act_func_set_id is a positional index), but hide Exp/Ln/Relu/Copy/Identity
# from all other sets so the whole kernel uses the one combined set (1 load).
_orig_get_tables = bacc.get_activation_tables


def _tables_lnexp_first(arch):
    t = _orig_get_tables(arch)
    pref = "natural_log_exp_and_others"
    if pref not in t:
        return t
    hide = {AF.Exp, AF.Ln, AF.Relu, AF.Copy, AF.Identity}
    out = {}
    for k, v in t.items():
        if k != pref and (v & hide):
            v = v - hide
        out[k] = v
    return out


bacc.get_activation_tables = _tables_lnexp_first

N_CORES = 8
B_FULL, COV, H, K = 16384, 256, 512, 8
B = B_FULL // N_CORES  # 2048 per core
TT, TS = 4, 512        # batch column tiles
C = H // 128           # 4 H-chunks
CIN = COV // 128       # 2 cov-chunks

# --- tuning knobs -----------------------------------------------------------
NQ = 6                 # quadrature nodes (ref=15; 6 -> 7e-5 quad error)
PE_ROUTE = 10          # of NQ*C h units, how many go PE+evac (rest DVE stt)
PE_EVAC_DVE = 0        # of the PE-route units, how many evac on DVE (rest ACT)
RELU_ACT = 0           # of the DVE-route units, how many relu on ACT
RELU_GPS = 0           # of the DVE-route units, how many relu on GPSIMD (slow!)
G_EVAC_ACT = 8         # of the 16 g evacs, how many on ACT (rest DVE)
H1P_EVAC_DVE = 0       # of the 16 h1p evacs, how many on DVE (rest ACT)
# ---------------------------------------------------------------------------

_u64, _w64 = np.polynomial.legendre.leggauss(NQ)
CN = [float(np.float32(0.5) * (np.float32(1.0) + u)) for u in _u64.astype(np.float32)]
WN = [float(w) for w in _w64.astype(np.float32)]

# layer-2 strip groups: chunks of up to 4 nodes; pi head rides in the last
# group's strip 3 (or its own group if the last one is full).
_node_groups = [list(range(i, min(i + 4, NQ))) for i in range(0, NQ, 4)]
if len(_node_groups[-1]) <= 3:
    PI_GROUP = len(_node_groups) - 1
else:
    _node_groups.append([])
    PI_GROUP = len(_node_groups) - 1
PI_STRIP = 3
NGROUPS = len(_node_groups)


def _spread(n_total, count):
    return {i for i in range(n_total) if ((i + 1) * count) // n_total > (i * count) // n_total}


def build_kernel():
    nc = bacc.Bacc("TRN2", target_bir_lowering=False, debug=False)

    x_d = nc.dram_tensor("x", [B, COV], F32, kind="ExternalInput").ap()
    t_d = nc.dram_tensor("t", [B], F32, kind="ExternalInput").ap()
    w1p_d = nc.dram_tensor("W1p", [COV, H], F32, kind="ExternalInput").ap()
    b1p_d = nc.dram_tensor("b1p", [H], F32, kind="ExternalInput").ap()
    w2p_d = nc.dram_tensor("W2p", [H, K], F32, kind="ExternalInput").ap()
    b2p_d = nc.dram_tensor("b2p", [K], F32, kind="ExternalInput").ap()
    w1o_d = nc.dram_tensor("W1o", [COV + 1, H], F32, kind="ExternalInput").ap()
    b1o_d = nc.dram_tensor("b1o", [H], F32, kind="ExternalInput").ap()
    w2o_d = nc.dram_tensor("W2o", [H, K], F32, kind="ExternalInput").ap()
    b2o_d = nc.dram_tensor("b2o", [K], F32, kind="ExternalInput").ap()
    preds_d = nc.dram_tensor("preds", [B, K], F32, kind="ExternalOutput").ap()
    pi_d = nc.dram_tensor("pi", [B, K], F32, kind="ExternalOutput").ap()

    n_units = NQ * C
    pe_units = _spread(n_units, PE_ROUTE)
    pe_units_l = sorted(pe_units)
    pe_evac_dve = {pe_units_l[i] for i in sorted(_spread(len(pe_units_l), PE_EVAC_DVE))} if pe_units_l else set()
    dve_units_l = sorted(set(range(n_units)) - pe_units)
    relu_act = {dve_units_l[i] for i in sorted(_spread(len(dve_units_l), RELU_ACT))} if dve_units_l else set()
    rest_l = [i for i in dve_units_l if i not in relu_act]
    relu_gps = {rest_l[i] for i in sorted(_spread(len(rest_l), RELU_GPS))} if rest_l else set()
    g_evac_act = _spread(16, G_EVAC_ACT)
    h1p_evac_dve = _spread(16, H1P_EVAC_DVE)

    with tile.TileContext(nc) as tc:
        with (
            tc.tile_pool(name="pers", bufs=1) as pers,
            tc.tile_pool(name="ph", bufs=n_units) as ph,
            tc.tile_pool(name="pxin", bufs=3) as pxin,
            tc.tile_pool(name="pft", bufs=4) as pft,
            tc.tile_pool(name="psm", bufs=1) as psm,
            tc.tile_pool(name="pps", bufs=4, space="PSUM") as pps,
            tc.tile_pool(name="ppsf", bufs=2, space="PSUM") as ppsf,
            tc.tile_pool(name="ppred", bufs=2, space="PSUM") as ppred,
        ):
            def pt(name, shape, dt=F32):
                return pers.tile(shape, dt, tag=name, name=name)

            # ---- persistent SBUF tiles ----
            ident128 = pt("ident128", [128, 128])          # fp32, for x transpose
            identB = pt("identB", [128, 128], BF)          # bf16, for PE h route
            ident8 = pt("ident8", [8, 8])                  # fp32, small transposes
            xT = pt("xT", [128, CIN * B], BF)              # [128, ci*2048+b]
            g_sb = [pt(f"g{c}", [128, B], BF) for c in range(C)]
            h1p_sb = [pt(f"h1p{c}", [128, B], BF) for c in range(C)]
            t_bcast = pt("t_bcast", [128, B], BF)
            t_row_bf = pt("t_row_bf", [1, B], BF)
            ones_row = pt("ones_row", [1, 128], BF)
            w1o_sb = [pt(f"w1o{ci}", [128, H], BF) for ci in range(CIN)]
            w1p_sb = [pt(f"w1p{ci}", [128, H], BF) for ci in range(CIN)]
            w2o_sb = [pt(f"w2o{c}", [128, K], BF) for c in range(C)]
            w2p_sb = [pt(f"w2p{c}", [128, K], BF) for c in range(C)]
            w_row = pt("w_row", [1, H])                    # fp32 W1o[-1]
            w_pc = pt("w_pc", [128, C])                    # fp32 W1o[-1] as [p,c]
            wsc_row = [pt(f"wscr{n}", [1, H], BF) for n in range(NQ)]
            wsc_pc = [pt(f"wscp{n}", [128, C], BF) for n in range(NQ)]
            b1o_pc = pt("b1o_pc", [128, C])
            b1p_pc = pt("b1p_pc", [128, C])
            b2o_col = pt("b2o_col", [128, 1])
            b2p_col = pt("b2p_col", [128, 1])
            R_sb = [pt(f"R{g}", [128, 8], BF) for g in range(NGROUPS)]
            pred_sb = pt("pred_sb", [8, B])
            lgt_sb = pt("lgt_sb", [8, B])
            pred_b = pt("pred_b", [128, B // 128 * K])
            logits_b = pt("logits_b", [128, B // 128 * K])
            e_b = pt("e_b", [128, B // 128 * K])
            eneg = pt("eneg", [128, B // 128 * K])
            sums = pt("sums", [128, B // 128])
            rec = pt("rec", [128, B // 128])
            pi_b = pt("pi_b", [128, B // 128 * K])
            cif_b = pt("cif_b", [128, B // 128 * K])
            preds_b = pt("preds_b", [128, B // 128 * K])

            # ---- constants ----
            make_identity(nc, ident128)
            make_identity(nc, identB)
            make_identity(nc, ident8)
            nc.vector.memset(ones_row, 1.0)
            nc.vector.memset(b2o_col, 0.0)
            nc.vector.memset(b2p_col, 0.0)

            # ---- weight / small input DMAs + bf16 casts ----
            for ci in range(CIN):
                w1o_ld = psm.tile([128, H], F32, tag="wld", name=f"w1old{ci}")
                nc.sync.dma_start(out=w1o_ld, in_=w1o_d[ci * 128 : (ci + 1) * 128, :])
                nc.vector.tensor_copy(w1o_sb[ci], w1o_ld)
                w1p_ld = psm.tile([128, H], F32, tag="wld2", name=f"w1pld{ci}")
                nc.sync.dma_start(out=w1p_ld, in_=w1p_d[ci * 128 : (ci + 1) * 128, :])
                nc.vector.tensor_copy(w1p_sb[ci], w1p_ld)
            for c in range(C):
                w2o_ld = psm.tile([128, K], F32, tag="w2ld", name=f"w2old{c}")
                nc.sync.dma_start(out=w2o_ld, in_=w2o_d[c * 128 : (c + 1) * 128, :])
                nc.vector.tensor_copy(w2o_sb[c], w2o_ld)
                w2p_ld = psm.tile([128, K], F32, tag="w2ld2", name=f"w2pld{c}")
                nc.sync.dma_start(out=w2p_ld, in_=w2p_d[c * 128 : (c + 1) * 128, :])
                nc.vector.tensor_copy(w2p_sb[c], w2p_ld)

            t_row_ld = pers.tile([1, B], F32, tag="trow", name="t_row_ld")
            nc.sync.dma_start(out=t_row_ld, in_=t_d.rearrange("(a b) -> a b", a=1))
            nc.vector.tensor_copy(t_row_bf, t_row_ld)
            nc.sync.dma_start(out=w_row, in_=w1o_d[COV : COV + 1, :])
            nc.sync.dma_start(
                out=w_pc, in_=w1o_d[COV : COV + 1, :].rearrange("a (c p) -> p (c a)", p=128)
            )
            for n in range(NQ):
                nc.vector.tensor_scalar_mul(wsc_row[n], w_row, CN[n])
                nc.vector.tensor_scalar_mul(wsc_pc[n], w_pc, CN[n])
            nc.sync.dma_start(out=b1o_pc, in_=b1o_d.rearrange("(c p) -> p c", p=128))
            nc.sync.dma_start(out=b1p_pc, in_=b1p_d.rearrange("(c p) -> p c", p=128))
            for j in range(4):
                nc.sync.dma_start(
                    out=b2o_col[32 * j : 32 * j + 8, :],
                    in_=b2o_d.rearrange("(k a) -> k a", a=1),
                )
            nc.sync.dma_start(
                out=b2p_col[32 * PI_STRIP : 32 * PI_STRIP + 8, :],
                in_=b2p_d.rearrange("(k a) -> k a", a=1),
            )
            # strip-weight matrices: R[g][32j+k, k] = -0.5 * WN[node], else 0
            for g, nodes in enumerate(_node_groups):
                nc.vector.memset(R_sb[g], 0.0)
                for j, n in enumerate(nodes):
                    nc.scalar.activation(
                        R_sb[g][32 * j : 32 * j + 8, :], ident8, AF.Copy,
                        scale=-0.5 * WN[n],
                    )

            # ---- x load + transpose + cast to bf16 (feature-major xT) ----
            # 4 transposes (2 xin tiles x 2 cov chunks) batch into one psum
            # bank; one ACT copy evacuates them (cast to bf16) via a 4D AP.
            xT_v = xT.rearrange("p (ci b) -> p ci b", ci=CIN)
            for half in range(B // 256):
                pxt = pps.tile([128, 512], F32, tag="ps", name=f"pxt_{half}")
                for jj in range(2):
                    r = half * 256 + jj * 128
                    xin = pxin.tile([128, COV], F32, tag="xin", name=f"xin_{half}_{jj}")
                    # alternate the two HWDGE queues (sync + scalar) so the
                    # 2MB x load doesn't serialize on one queue
                    dma_eng = nc.sync if (half * 2 + jj) % 2 == 0 else nc.scalar
                    dma_eng.dma_start(out=xin, in_=x_d[r : r + 128, :])
                    for ci in range(CIN):
                        nc.tensor.transpose(
                            pxt[:, (jj * 2 + ci) * 128 : (jj * 2 + ci + 1) * 128],
                            xin[:, ci * 128 : (ci + 1) * 128],
                            ident128,
                        )
                nc.scalar.copy(
                    xT_v[:, :, half * 256 : (half + 1) * 256].rearrange(
                        "p ci (jj q) -> p jj ci q", jj=2
                    ),
                    pxt.rearrange("p (jj ci q) -> p jj ci q", jj=2, ci=CIN),
                )

            # ---- t_bcast[p, b] = t[b] (rank-1 ones x t) ----
            for T in range(TT):
                bs = slice(T * TS, (T + 1) * TS)
                pst = pps.tile([128, TS], F32, tag="ps", name=f"ptb_{T}")
                nc.tensor.matmul(pst, ones_row, t_row_bf[:, bs], start=True, stop=True)
                nc.vector.tensor_copy(t_bcast[:, bs], pst)

            # ---- layer-1 matmuls (both nets) ----
            for c in range(C):
                cs = slice(c * 128, (c + 1) * 128)
                for T in range(TT):
                    bs = slice(T * TS, (T + 1) * TS)
                    i = c * TT + T
                    pso = pps.tile([128, TS], F32, tag="ps", name=f"pso_{c}_{T}")
                    for ci in range(CIN):
                        nc.tensor.matmul(
                            pso, w1o_sb[ci][:, cs],
                            xT_v[:, ci, T * TS : (T + 1) * TS],
                            start=(ci == 0), stop=(ci == CIN - 1),
                        )
                    if i in g_evac_act:
                        nc.scalar.activation(
                            g_sb[c][:, bs], pso, AF.Identity, bias=b1o_pc[:, c : c + 1]
                        )
                    else:
                        nc.vector.tensor_scalar_add(
                            g_sb[c][:, bs], pso, b1o_pc[:, c : c + 1]
                        )
                    psp = pps.tile([128, TS], F32, tag="ps", name=f"psp_{c}_{T}")
                    for ci in range(CIN):
                        nc.tensor.matmul(
                            psp, w1p_sb[ci][:, cs],
                            xT_v[:, ci, T * TS : (T + 1) * TS],
                            start=(ci == 0), stop=(ci == CIN - 1),
                        )
                    if i in h1p_evac_dve:
                        nc.vector.tensor_scalar(
                            h1p_sb[c][:, bs], psp, b1p_pc[:, c : c + 1], 0.0,
                            OP.add, OP.max,
                        )
                    else:
                        nc.scalar.activation(
                            h1p_sb[c][:, bs], psp, AF.Relu, bias=b1p_pc[:, c : c + 1]
                        )

            # ---- h units ----
            h_tiles = {}
            for n in range(NQ):
                for c in range(C):
                    i = n * C + c
                    ht = ph.tile([128, B], BF, tag="h", name=f"h_{n}_{c}")
                    if i in pe_units:
                        cs = slice(c * 128, (c + 1) * 128)
                        # group the identity matmuls then the rank-1s so the
                        # stationary operand only reloads twice per unit
                        pshs = []
                        for T in range(TT):
                            bs = slice(T * TS, (T + 1) * TS)
                            psh = pps.tile([128, TS], F32, tag="ps", name=f"psh_{n}_{c}_{T}")
                            nc.tensor.matmul(psh, identB, g_sb[c][:, bs], start=True, stop=False)
                            pshs.append(psh)
                        for T in range(TT):
                            bs = slice(T * TS, (T + 1) * TS)
                            nc.tensor.matmul(
                                pshs[T], wsc_row[n][:, cs], t_row_bf[:, bs],
                                start=False, stop=True,
                            )
                        for T in range(TT):
                            bs = slice(T * TS, (T + 1) * TS)
                            if i in pe_evac_dve:
                                nc.vector.tensor_scalar_max(ht[:, bs], pshs[T], 0.0)
                            else:
                                nc.scalar.activation(ht[:, bs], pshs[T], AF.Relu)
                    else:
                        nc.vector.scalar_tensor_tensor(
                            out=ht, in0=t_bcast, scalar=wsc_pc[n][:, c : c + 1],
                            in1=g_sb[c], op0=OP.mult, op1=OP.add,
                        )
                        if i in relu_act:
                            nc.scalar.activation(ht, ht, AF.Relu)
                        elif i in relu_gps:
                            nc.gpsimd.tensor_scalar_max(ht, ht, 0.0)
                        else:
                            nc.vector.tensor_scalar_max(ht, ht, 0.0)
                    h_tiles[(n, c)] = ht

            # ---- layer-2 (col-tiled strips) + softplus + quadrature,
            #      T-major so the tail work of early T overlaps later T ----
            fgroups = [g for g, nodes in enumerate(_node_groups) if nodes]
            psf_count = 0
            for T in range(TT):
                bs = slice(T * TS, (T + 1) * TS)
                f_tiles = {}
                for g, nodes in enumerate(_node_groups):
                    psf = ppsf.tile([128, TS], F32, tag="psf", name=f"psf_{g}_{T}")
                    if psf_count < 2:
                        # first touch of each ring slot: clear garbage rows so
                        # exp of unwritten partitions stays finite
                        nc.vector.memset(psf, 0.0)
                    psf_count += 1
                    for c in range(C):
                        for j, n in enumerate(nodes):
                            nc.tensor.matmul(
                                psf[32 * j : 32 * j + 8, :],
                                w2o_sb[c], h_tiles[(n, c)][:, bs],
                                start=(c == 0), stop=(c == C - 1),
                                tile_position=(0, 32 * j),
                            )
                        if g == PI_GROUP:
                            nc.tensor.matmul(
                                psf[32 * PI_STRIP : 32 * PI_STRIP + 8, :],
                                w2p_sb[c], h1p_sb[c][:, bs],
                                start=(c == 0), stop=(c == C - 1),
                                tile_position=(0, 32 * PI_STRIP),
                            )
                    if nodes:
                        top = 32 * (len(nodes) - 1) + 8
                        et = pft.tile([top, TS], F32, tag="et", name=f"et_{g}_{T}")
                        nc.scalar.activation(
                            et, psf[0:top, :], AF.Exp, bias=b2o_col[0:top, :]
                        )
                        ft = pft.tile([top, TS], BF, tag="ft", name=f"ft_{g}_{T}")
                        nc.scalar.activation(ft, et, AF.Ln, bias=1.0)
                        f_tiles[g] = ft
                    if g == PI_GROUP:
                        nc.vector.tensor_scalar_add(
                            lgt_sb[:, bs],
                            psf[32 * PI_STRIP : 32 * PI_STRIP + 8, :],
                            b2p_col[32 * PI_STRIP : 32 * PI_STRIP + 8, :],
                        )

                # quadrature: pred_sb = -(t/2) sum_n W_n f_n
                pp = ppred.tile([8, TS], F32, tag="ppred", name=f"ppred_{T}")
                for k, g in enumerate(fgroups):
                    top = 32 * (len(_node_groups[g]) - 1) + 8
                    nc.tensor.matmul(
                        pp, R_sb[g][0:top, :], f_tiles[g],
                        start=(k == 0), stop=(k == len(fgroups) - 1),
                    )
                # fold in the per-column t factor during evacuation
                nc.vector.scalar_tensor_tensor(
                    out=pred_sb[:, bs], in0=pp, scalar=1.0,
                    in1=t_bcast[0:8, bs], op0=OP.mult, op1=OP.mult,
                )

                # transpose pred + logits to batch-major
                pxp = pps.tile([128, 32], F32, tag="ps", name=f"pxp_{T}")
                pxl = pps.tile([128, 32], F32, tag="ps", name=f"pxl_{T}")
                for j in range(4):
                    off = T * TS + j * 128
                    nc.tensor.transpose(
                        pxp[:, j * 8 : (j + 1) * 8], pred_sb[:, off : off + 128], ident8
                    )
                    nc.tensor.transpose(
                        pxl[:, j * 8 : (j + 1) * 8], lgt_sb[:, off : off + 128], ident8
                    )
                nc.vector.tensor_copy(pred_b[:, T * 32 : (T + 1) * 32], pxp)
                nc.vector.tensor_copy(logits_b[:, T * 32 : (T + 1) * 32], pxl)

            # ---- final: softmax(pi), cif, preds (table switch to exp here) ----
            nc.scalar.activation(e_b, logits_b, AF.Exp)
            nc.scalar.activation(eneg, pred_b, AF.Exp)
            nc.vector.tensor_reduce(
                sums, e_b.rearrange("p (t k) -> p t k", k=8), axis=AX.X, op=OP.add
            )
            nc.vector.reciprocal(rec, sums)
            for jj in range(B // 128):
                nc.vector.tensor_scalar_mul(
                    pi_b[:, jj * 8 : (jj + 1) * 8],
                    e_b[:, jj * 8 : (jj + 1) * 8],
                    rec[:, jj : jj + 1],
                )
            nc.vector.tensor_scalar(cif_b, eneg, -1.0, 1.0, OP.mult, OP.add)
            nc.vector.tensor_tensor(out=preds_b, in0=cif_b, in1=pi_b, op=OP.mult)

            nc.sync.dma_start(
                out=preds_d.rearrange("(j p) k -> p j k", p=128),
                in_=preds_b.rearrange("p (j k) -> p j k", k=8),
            )
            nc.sync.dma_start(
                out=pi_d.rearrange("(j p) k -> p j k", p=128),
                in_=pi_b.rearrange("p (j k) -> p j k", k=8),
            )

    nc.compile()
    return nc


_NC = None


def _get_nc():
    global _NC
    if _NC is None:
        _NC = build_kernel()
    return _NC


def _shard_inputs(inputs):
    in_maps = []
    for i in range(N_CORES):
        sl = slice(i * B, (i + 1) * B)
        m = {
            "x": np.ascontiguousarray(np.asarray(inputs["x"], np.float32)[sl]),
            "t": np.ascontiguousarray(np.asarray(inputs["t"], np.float32)[sl]),
        }
        for k in ("W1p", "b1p", "W2p", "b2p", "W1o", "b1o", "W2o", "b2o"):
            m[k] = np.asarray(inputs[k], np.float32)
        in_maps.append(m)
    return in_maps


def kernel(**inputs):
    nc = _get_nc()
    in_maps = _shard_inputs(inputs)
    res = run_bass_kernel_spmd(nc, in_maps, core_ids=list(range(N_CORES)))
    preds = np.concatenate([res.results[i]["preds"] for i in range(N_CORES)], axis=0)
    pi = np.concatenate([res.results[i]["pi"] for i in range(N_CORES)], axis=0)
    return (preds, pi)


# revision 20
# speedup vs baseline: 1.9748x; 1.1971x over previous
"""Trainium2 Bass kernel for nn_ODESurvMultiple (dense_mlp, 8-core data parallel).

reference math (per sample row x[256], scalar t):
  pi    = softmax(relu(x@W1p+b1p) @ W2p + b2p)                      [K=8]
  g     = x @ W1o[:-1] + b1o                                        [H=512]
  h_n   = relu(g + c_n * (t * w))     c_n=(1+u_n)/2, w=W1o[-1]      [NQ, 512]
  f_n   = softplus(h_n @ W2o + b2o)                                 [NQ, 8]
  pred  = (t/2) * sum_n W_n f_n                                     [8]
  preds = pi * (1 - exp(-pred))
returns (preds, pi)

Implementation notes:
- NQ quadrature nodes (reference uses 15; Gauss-Legendre converges so fast on
  this integrand that NQ=6 matches the 15-node reference to ~7e-5, far inside
  the 2e-2 gate; bf16 rounding dominates the error at ~4e-3).
- bf16 operands everywhere on the PE; fp32 PSUM accumulation.
- layer-2 packs up to 4 quadrature nodes (and the pi-head logits) into one
  PSUM tile via column tile_position strips -> 4 concurrent matmuls, and the
  softplus + quadrature sum then run at 104-partition width instead of 8.
- softplus is a single ACT pass (softplus_and_others table also has relu).
- quadrature sum is a PE matmul against a strip-weight matrix R with
  -W_n/2 folded in; the (t) factor is applied during the psum evacuation.
- h build: mix of DVE route (stt: t_bcast*w_pc + g, then relu) and PE route
  (identity re-inject + rank-1 into psum, relu-evac on ACT/DVE).
"""

import os
import sys

for _p in (
    "/root/.axon_site",
    "/root/.axon_site/_ro/trn_rl_repo",
    "/root/.axon_site/_ro/pypackages",
    "/opt/trn_rl_repo",
):
    if os.path.isdir(_p) and _p not in sys.path:
        sys.path.append(_p)

import numpy as np

import concourse.bass as bass
import concourse.mybir as mybir
import concourse.tile as tile
from concourse import bacc
from concourse.bass_utils import run_bass_kernel_spmd
from concourse.masks import make_identity

F32 = mybir.dt.float32
BF = mybir.dt.bfloat16
AX = mybir.AxisListType
OP = mybir.AluOpType
AF = mybir.ActivationFunctionType

# Steer the greedy act-table selector: keep set ORDER identical (the emitted
# act_func_set_id is a positional index), but hide Exp/Ln/Relu/Copy/Identity
# from all other sets so the whole kernel uses the one combined set (1 load).
_orig_get_tables = bacc.get_activation_tables


def _tables_lnexp_first(arch):
    t = _orig_get_tables(arch)
    pref = "natural_log_exp_and_others"
    if pref not in t:
        return t
    hide = {AF.Exp, AF.Ln, AF.Relu, AF.Copy, AF.Identity}
    out = {}
    for k, v in t.items():
        if k != pref and (v & hide):
            v = v - hide
        out[k] = v
    return out


bacc.get_activation_tables = _tables_lnexp_first

N_CORES = 8
B_FULL, COV, H, K = 16384, 256, 512, 8
B = B_FULL // N_CORES  # 2048 per core
TT, TS = 4, 512        # batch column tiles
C = H // 128           # 4 H-chunks
CIN = COV // 128       # 2 cov-chunks

# --- tuning knobs -----------------------------------------------------------
NQ = 5                 # quadrature nodes (ref=15; 5 -> 1e-4 quad error)
PE_ROUTE = 6           # of NQ*C h units, how many go PE+evac (rest DVE stt)
PE_EVAC_DVE = 0        # of the PE-route units, how many evac on DVE (rest ACT)
RELU_ACT = 0           # of the DVE-route units, how many relu on ACT
RELU_GPS = 0           # of the DVE-route units, how many relu on GPSIMD (slow!)
G_EVAC_ACT = 8         # of the 16 g evacs, how many on ACT (rest DVE)
H1P_EVAC_DVE = 0       # of the 16 h1p evacs, how many on DVE (rest ACT)
# ---------------------------------------------------------------------------

_u64, _w64 = np.polynomial.legendre.leggauss(NQ)
CN = [float(np.float32(0.5) * (np.float32(1.0) + u)) for u in _u64.astype(np.float32)]
WN = [float(w) for w in _w64.astype(np.float32)]

# layer-2 strip groups: chunks of up to 4 nodes; pi head rides in the last
# group's strip 3 (or its own group if the last one is full).
_node_groups = [list(range(i, min(i + 4, NQ))) for i in range(0, NQ, 4)]
if len(_node_groups[-1]) <= 3:
    PI_GROUP = len(_node_groups) - 1
else:
    _node_groups.append([])
    PI_GROUP = len(_node_groups) - 1
PI_STRIP = 3
NGROUPS = len(_node_groups)


def _spread(n_total, count):
    return {i for i in range(n_total) if ((i + 1) * count) // n_total > (i * count) // n_total}


def build_kernel():
    nc = bacc.Bacc("TRN2", target_bir_lowering=False, debug=False)

    x_d = nc.dram_tensor("x", [B, COV], F32, kind="ExternalInput").ap()
    t_d = nc.dram_tensor("t", [B], F32, kind="ExternalInput").ap()
    w1p_d = nc.dram_tensor("W1p", [COV, H], F32, kind="ExternalInput").ap()
    b1p_d = nc.dram_tensor("b1p", [H], F32, kind="ExternalInput").ap()
    w2p_d = nc.dram_tensor("W2p", [H, K], F32, kind="ExternalInput").ap()
    b2p_d = nc.dram_tensor("b2p", [K], F32, kind="ExternalInput").ap()
    w1o_d = nc.dram_tensor("W1o", [COV + 1, H], F32, kind="ExternalInput").ap()
    b1o_d = nc.dram_tensor("b1o", [H], F32, kind="ExternalInput").ap()
    w2o_d = nc.dram_tensor("W2o", [H, K], F32, kind="ExternalInput").ap()
    b2o_d = nc.dram_tensor("b2o", [K], F32, kind="ExternalInput").ap()
    preds_d = nc.dram_tensor("preds", [B, K], F32, kind="ExternalOutput").ap()
    pi_d = nc.dram_tensor("pi", [B, K], F32, kind="ExternalOutput").ap()

    n_units = NQ * C
    pe_units = _spread(n_units, PE_ROUTE)
    pe_units_l = sorted(pe_units)
    pe_evac_dve = {pe_units_l[i] for i in sorted(_spread(len(pe_units_l), PE_EVAC_DVE))} if pe_units_l else set()
    dve_units_l = sorted(set(range(n_units)) - pe_units)
    relu_act = {dve_units_l[i] for i in sorted(_spread(len(dve_units_l), RELU_ACT))} if dve_units_l else set()
    rest_l = [i for i in dve_units_l if i not in relu_act]
    relu_gps = {rest_l[i] for i in sorted(_spread(len(rest_l), RELU_GPS))} if rest_l else set()
    g_evac_act = _spread(16, G_EVAC_ACT)
    h1p_evac_dve = _spread(16, H1P_EVAC_DVE)

    with tile.TileContext(nc) as tc:
        with (
            tc.tile_pool(name="pers", bufs=1) as pers,
            tc.tile_pool(name="ph", bufs=n_units) as ph,
            tc.tile_pool(name="pxin", bufs=2) as pxin,
            tc.tile_pool(name="pft", bufs=4) as pft,
            tc.tile_pool(name="psm", bufs=1) as psm,
            tc.tile_pool(name="pps", bufs=4, space="PSUM") as pps,
            tc.tile_pool(name="ppsf", bufs=2, space="PSUM") as ppsf,
            tc.tile_pool(name="ppred", bufs=2, space="PSUM") as ppred,
        ):
            def pt(name, shape, dt=F32):
                return pers.tile(shape, dt, tag=name, name=name)

            # ---- persistent SBUF tiles ----
            ident128 = pt("ident128", [128, 128])          # fp32, for x transpose
            identB = pt("identB", [128, 128], BF)          # bf16, for PE h route
            ident8 = pt("ident8", [8, 8])                  # fp32, small transposes
            xT = pt("xT", [128, CIN * B], BF)              # [128, ci*2048+b]
            g_sb = [pt(f"g{c}", [128, B], BF) for c in range(C)]
            h1p_sb = [pt(f"h1p{c}", [128, B], BF) for c in range(C)]
            t_bcast = pt("t_bcast", [128, B], BF)
            t_row_bf = pt("t_row_bf", [1, B], BF)
            ones_row = pt("ones_row", [1, 128], BF)
            w1o_sb = [pt(f"w1o{ci}", [128, H], BF) for ci in range(CIN)]
            w1p_sb = [pt(f"w1p{ci}", [128, H], BF) for ci in range(CIN)]
            w2o_sb = [pt(f"w2o{c}", [128, K], BF) for c in range(C)]
            w2p_sb = [pt(f"w2p{c}", [128, K], BF) for c in range(C)]
            w_row = pt("w_row", [1, H])                    # fp32 W1o[-1]
            w_pc = pt("w_pc", [128, C])                    # fp32 W1o[-1] as [p,c]
            wsc_row = [pt(f"wscr{n}", [1, H], BF) for n in range(NQ)]
            wsc_pc = [pt(f"wscp{n}", [128, C], BF) for n in range(NQ)]
            b1o_pc = pt("b1o_pc", [128, C])
            b1p_pc = pt("b1p_pc", [128, C])
            b2o_col = pt("b2o_col", [128, 1])
            b2p_col = pt("b2p_col", [128, 1])
            R_sb = [pt(f"R{g}", [128, 8], BF) for g in range(NGROUPS)]
            pred_sb = pt("pred_sb", [8, B])
            lgt_sb = pt("lgt_sb", [8, B])
            pred_b = pt("pred_b", [128, B // 128 * K])
            logits_b = pt("logits_b", [128, B // 128 * K])
            e_b = pt("e_b", [128, B // 128 * K])
            eneg = pt("eneg", [128, B // 128 * K])
            sums = pt("sums", [128, B // 128])
            rec = pt("rec", [128, B // 128])
            pi_b = pt("pi_b", [128, B // 128 * K])
            cif_b = pt("cif_b", [128, B // 128 * K])
            preds_b = pt("preds_b", [128, B // 128 * K])

            # ---- constants ----
            make_identity(nc, ident128)
            make_identity(nc, identB)
            make_identity(nc, ident8)
            nc.vector.memset(ones_row, 1.0)
            nc.vector.memset(b2o_col, 0.0)
            nc.vector.memset(b2p_col, 0.0)

            # ---- weight / small input DMAs + bf16 casts ----
            # consolidate into few big transfers: one dma_start spreads across
            # all 16 SDMA engines, and each trigger costs ~600ns of queue time
            w1o_ld = psm.tile([128, CIN * H], F32, tag="wld", name="w1old")
            nc.sync.dma_start(
                out=w1o_ld.rearrange("p (ci q) -> p ci q", ci=CIN),
                in_=w1o_d[0:COV, :].rearrange("(ci p) q -> p ci q", p=128),
            )
            for ci in range(CIN):
                nc.vector.tensor_copy(w1o_sb[ci], w1o_ld[:, ci * H : (ci + 1) * H])
            w1p_ld = psm.tile([128, CIN * H], F32, tag="wld2", name="w1pld")
            nc.sync.dma_start(
                out=w1p_ld.rearrange("p (ci q) -> p ci q", ci=CIN),
                in_=w1p_d.rearrange("(ci p) q -> p ci q", p=128),
            )
            for ci in range(CIN):
                nc.vector.tensor_copy(w1p_sb[ci], w1p_ld[:, ci * H : (ci + 1) * H])
            w2o_ld = psm.tile([128, C * K], F32, tag="w2ld", name="w2old")
            nc.scalar.dma_start(
                out=w2o_ld.rearrange("p (c k) -> p c k", c=C),
                in_=w2o_d.rearrange("(c p) k -> p c k", p=128),
            )
            w2p_ld = psm.tile([128, C * K], F32, tag="w2ld2", name="w2pld")
            nc.scalar.dma_start(
                out=w2p_ld.rearrange("p (c k) -> p c k", c=C),
                in_=w2p_d.rearrange("(c p) k -> p c k", p=128),
            )
            for c in range(C):
                nc.vector.tensor_copy(w2o_sb[c], w2o_ld[:, c * K : (c + 1) * K])
                nc.vector.tensor_copy(w2p_sb[c], w2p_ld[:, c * K : (c + 1) * K])

            t_row_ld = pers.tile([1, B], F32, tag="trow", name="t_row_ld")
            nc.scalar.dma_start(out=t_row_ld, in_=t_d.rearrange("(a b) -> a b", a=1))
            nc.vector.tensor_copy(t_row_bf, t_row_ld)
            nc.scalar.dma_start(out=w_row, in_=w1o_d[COV : COV + 1, :])
            nc.scalar.dma_start(
                out=w_pc, in_=w1o_d[COV : COV + 1, :].rearrange("a (c p) -> p (c a)", p=128)
            )
            for n in range(NQ):
                nc.vector.tensor_scalar_mul(wsc_row[n], w_row, CN[n])
                nc.vector.tensor_scalar_mul(wsc_pc[n], w_pc, CN[n])
            nc.scalar.dma_start(out=b1o_pc, in_=b1o_d.rearrange("(c p) -> p c", p=128))
            nc.scalar.dma_start(out=b1p_pc, in_=b1p_d.rearrange("(c p) -> p c", p=128))
            for j in range(4):
                nc.scalar.dma_start(
                    out=b2o_col[32 * j : 32 * j + 8, :],
                    in_=b2o_d.rearrange("(k a) -> k a", a=1),
                )
            nc.scalar.dma_start(
                out=b2p_col[32 * PI_STRIP : 32 * PI_STRIP + 8, :],
                in_=b2p_d.rearrange("(k a) -> k a", a=1),
            )
            # strip-weight matrices: R[g][32j+k, k] = -0.5 * WN[node], else 0
            for g, nodes in enumerate(_node_groups):
                nc.vector.memset(R_sb[g], 0.0)
                for j, n in enumerate(nodes):
                    nc.scalar.activation(
                        R_sb[g][32 * j : 32 * j + 8, :], ident8, AF.Copy,
                        scale=-0.5 * WN[n],
                    )

            # ---- x load + transpose + cast to bf16 (feature-major xT) ----
            # x comes in 4 big DMAs of 512 rows each ([128, 4, 256] row-
            # interleaved); 4 transposes batch into one psum bank and one ACT
            # copy evacuates them (cast to bf16) via a 4D AP.
            xT_v = xT.rearrange("p (ci b) -> p ci b", ci=CIN)
            for blk in range(4):
                xin = pxin.tile([128, 4 * COV], F32, tag="xin", name=f"xin_{blk}")
                dma_eng = nc.sync if blk % 2 == 0 else nc.scalar
                dma_eng.dma_start(
                    out=xin.rearrange("p (q c) -> p q c", q=4),
                    in_=x_d[blk * 512 : (blk + 1) * 512, :].rearrange(
                        "(q p) c -> p q c", p=128
                    ),
                )
                for half in range(2):
                    pxt = pps.tile([128, 512], F32, tag="ps", name=f"pxt_{blk}_{half}")
                    for jj in range(2):
                        q = half * 2 + jj
                        for ci in range(CIN):
                            nc.tensor.transpose(
                                pxt[:, (jj * 2 + ci) * 128 : (jj * 2 + ci + 1) * 128],
                                xin[:, q * COV + ci * 128 : q * COV + (ci + 1) * 128],
                                ident128,
                            )
                    r0 = blk * 512 + half * 256
                    nc.scalar.copy(
                        xT_v[:, :, r0 : r0 + 256].rearrange(
                            "p ci (jj q) -> p jj ci q", jj=2
                        ),
                        pxt.rearrange("p (jj ci q) -> p jj ci q", jj=2, ci=CIN),
                    )

            # ---- t_bcast[p, b] = t[b] (rank-1 ones x t) ----
            for T in range(TT):
                bs = slice(T * TS, (T + 1) * TS)
                pst = pps.tile([128, TS], F32, tag="ps", name=f"ptb_{T}")
                nc.tensor.matmul(pst, ones_row, t_row_bf[:, bs], start=True, stop=True)
                nc.vector.tensor_copy(t_bcast[:, bs], pst)

            # ---- layer-1 matmuls (both nets) ----
            for c in range(C):
                cs = slice(c * 128, (c + 1) * 128)
                for T in range(TT):
                    bs = slice(T * TS, (T + 1) * TS)
                    i = c * TT + T
                    pso = pps.tile([128, TS], F32, tag="ps", name=f"pso_{c}_{T}")
                    for ci in range(CIN):
                        nc.tensor.matmul(
                            pso, w1o_sb[ci][:, cs],
                            xT_v[:, ci, T * TS : (T + 1) * TS],
                            start=(ci == 0), stop=(ci == CIN - 1),
                        )
                    if i in g_evac_act:
                        nc.scalar.activation(
                            g_sb[c][:, bs], pso, AF.Identity, bias=b1o_pc[:, c : c + 1]
                        )
                    else:
                        nc.vector.tensor_scalar_add(
                            g_sb[c][:, bs], pso, b1o_pc[:, c : c + 1]
                        )
                    psp = pps.tile([128, TS], F32, tag="ps", name=f"psp_{c}_{T}")
                    for ci in range(CIN):
                        nc.tensor.matmul(
                            psp, w1p_sb[ci][:, cs],
                            xT_v[:, ci, T * TS : (T + 1) * TS],
                            start=(ci == 0), stop=(ci == CIN - 1),
                        )
                    if i in h1p_evac_dve:
                        nc.vector.tensor_scalar(
                            h1p_sb[c][:, bs], psp, b1p_pc[:, c : c + 1], 0.0,
                            OP.add, OP.max,
                        )
                    else:
                        nc.scalar.activation(
                            h1p_sb[c][:, bs], psp, AF.Relu, bias=b1p_pc[:, c : c + 1]
                        )

            # ---- h units ----
            h_tiles = {}
            for n in range(NQ):
                for c in range(C):
                    i = n * C + c
                    ht = ph.tile([128, B], BF, tag="h", name=f"h_{n}_{c}")
                    if i in pe_units:
                        cs = slice(c * 128, (c + 1) * 128)
                        # group the identity matmuls then the rank-1s so the
                        # stationary operand only reloads twice per unit
                        pshs = []
                        for T in range(TT):
                            bs = slice(T * TS, (T + 1) * TS)
                            psh = pps.tile([128, TS], F32, tag="ps", name=f"psh_{n}_{c}_{T}")
                            nc.tensor.matmul(psh, identB, g_sb[c][:, bs], start=True, stop=False)
                            pshs.append(psh)
                        for T in range(TT):
                            bs = slice(T * TS, (T + 1) * TS)
                            nc.tensor.matmul(
                                pshs[T], wsc_row[n][:, cs], t_row_bf[:, bs],
                                start=False, stop=True,
                            )
                        for T in range(TT):
                            bs = slice(T * TS, (T + 1) * TS)
                            if i in pe_evac_dve:
                                nc.vector.tensor_scalar_max(ht[:, bs], pshs[T], 0.0)
                            else:
                                nc.scalar.activation(ht[:, bs], pshs[T], AF.Relu)
                    else:
                        nc.vector.scalar_tensor_tensor(
                            out=ht, in0=t_bcast, scalar=wsc_pc[n][:, c : c + 1],
                            in1=g_sb[c], op0=OP.mult, op1=OP.add,
                        )
                        if i in relu_act:
                            nc.scalar.activation(ht, ht, AF.Relu)
                        elif i in relu_gps:
                            nc.gpsimd.tensor_scalar_max(ht, ht, 0.0)
                        else:
                            nc.vector.tensor_scalar_max(ht, ht, 0.0)
                    h_tiles[(n, c)] = ht

            # ---- layer-2 (col-tiled strips) + softplus + quadrature,
            #      T-major so the tail work of early T overlaps later T ----
            fgroups = [g for g, nodes in enumerate(_node_groups) if nodes]
            psf_count = 0
            for T in range(TT):
                bs = slice(T * TS, (T + 1) * TS)
                f_tiles = {}
                for g, nodes in enumerate(_node_groups):
                    psf = ppsf.tile([128, TS], F32, tag="psf", name=f"psf_{g}_{T}")
                    if psf_count < 2:
                        # first touch of each ring slot: clear garbage rows so
                        # exp of unwritten partitions stays finite
                        nc.vector.memset(psf, 0.0)
                    psf_count += 1
                    for c in range(C):
                        for j, n in enumerate(nodes):
                            nc.tensor.matmul(
                                psf[32 * j : 32 * j + 8, :],
                                w2o_sb[c], h_tiles[(n, c)][:, bs],
                                start=(c == 0), stop=(c == C - 1),
                                tile_position=(0, 32 * j),
                            )
                        if g == PI_GROUP:
                            nc.tensor.matmul(
                                psf[32 * PI_STRIP : 32 * PI_STRIP + 8, :],
                                w2p_sb[c], h1p_sb[c][:, bs],
                                start=(c == 0), stop=(c == C - 1),
                                tile_position=(0, 32 * PI_STRIP),
                            )
                    if nodes:
                        top = 32 * (len(nodes) - 1) + 8
                        et = pft.tile([top, TS], F32, tag="et", name=f"et_{g}_{T}")
                        nc.scalar.activation(
                            et, psf[0:top, :], AF.Exp, bias=b2o_col[0:top, :]
                        )
                        ft = pft.tile([top, TS], BF, tag="ft", name=f"ft_{g}_{T}")
                        nc.scalar.activation(ft, et, AF.Ln, bias=1.0)
                        f_tiles[g] = ft
                    if g == PI_GROUP:
                        nc.vector.tensor_scalar_add(
                            lgt_sb[:, bs],
                            psf[32 * PI_STRIP : 32 * PI_STRIP + 8, :],
                            b2p_col[32 * PI_STRIP : 32 * PI_STRIP + 8, :],
                        )

                # quadrature: pred_sb = -(t/2) sum_n W_n f_n
                pp = ppred.tile([8, TS], F32, tag="ppred", name=f"ppred_{T}")
                for k, g in enumerate(fgroups):
                    top = 32 * (len(_node_groups[g]) - 1) + 8
                    nc.tensor.matmul(
                        pp, R_sb[g][0:top, :], f_tiles[g],
                        start=(k == 0), stop=(k == len(fgroups) - 1),
                    )
                # fold in the per-column t factor during evacuation
                nc.vector.scalar_tensor_tensor(
                    out=pred_sb[:, bs], in0=pp, scalar=1.0,
                    in1=t_bcast[0:8, bs], op0=OP.mult, op1=OP.mult,
                )

                # transpose pred + logits to batch-major
                pxp = pps.tile([128, 32], F32, tag="ps", name=f"pxp_{T}")
                pxl = pps.tile([128, 32], F32, tag="ps", name=f"pxl_{T}")
                for j in range(4):
                    off = T * TS + j * 128
                    nc.tensor.transpose(
                        pxp[:, j * 8 : (j + 1) * 8], pred_sb[:, off : off + 128], ident8
                    )
                    nc.tensor.transpose(
                        pxl[:, j * 8 : (j + 1) * 8], lgt_sb[:, off : off + 128], ident8
                    )
                nc.vector.tensor_copy(pred_b[:, T * 32 : (T + 1) * 32], pxp)
                nc.vector.tensor_copy(logits_b[:, T * 32 : (T + 1) * 32], pxl)

            # ---- final: softmax(pi), cif, preds (table switch to exp here) ----
            nc.scalar.activation(e_b, logits_b, AF.Exp)
            nc.scalar.activation(eneg, pred_b, AF.Exp)
            nc.vector.tensor_reduce(
                sums, e_b.rearrange("p (t k) -> p t k", k=8), axis=AX.X, op=OP.add
            )
            nc.vector.reciprocal(rec, sums)
            for jj in range(B // 128):
                nc.vector.tensor_scalar_mul(
                    pi_b[:, jj * 8 : (jj + 1) * 8],
                    e_b[:, jj * 8 : (jj + 1) * 8],
                    rec[:, jj : jj + 1],
                )
            nc.vector.tensor_scalar(cif_b, eneg, -1.0, 1.0, OP.mult, OP.add)
            nc.vector.tensor_tensor(out=preds_b, in0=cif_b, in1=pi_b, op=OP.mult)

            nc.sync.dma_start(
                out=preds_d.rearrange("(j p) k -> p j k", p=128),
                in_=preds_b.rearrange("p (j k) -> p j k", k=8),
            )
            nc.sync.dma_start(
                out=pi_d.rearrange("(j p) k -> p j k", p=128),
                in_=pi_b.rearrange("p (j k) -> p j k", k=8),
            )

    nc.compile()
    return nc


_NC = None


def _get_nc():
    global _NC
    if _NC is None:
        _NC = build_kernel()
    return _NC


def _shard_inputs(inputs):
    in_maps = []
    for i in range(N_CORES):
        sl = slice(i * B, (i + 1) * B)
        m = {
            "x": np.ascontiguousarray(np.asarray(inputs["x"], np.float32)[sl]),
            "t": np.ascontiguousarray(np.asarray(inputs["t"], np.float32)[sl]),
        }
        for k in ("W1p", "b1p", "W2p", "b2p", "W1o", "b1o", "W2o", "b2o"):
            m[k] = np.asarray(inputs[k], np.float32)
        in_maps.append(m)
    return in_maps


def kernel(**inputs):
    nc = _get_nc()
    in_maps = _shard_inputs(inputs)
    res = run_bass_kernel_spmd(nc, in_maps, core_ids=list(range(N_CORES)))
    preds = np.concatenate([res.results[i]["preds"] for i in range(N_CORES)], axis=0)
    pi = np.concatenate([res.results[i]["pi"] for i in range(N_CORES)], axis=0)
    return (preds, pi)


# revision 26
# speedup vs baseline: 2.0362x; 1.0311x over previous
"""Trainium2 Bass kernel for nn_ODESurvMultiple (dense_mlp, 8-core data parallel).

reference math (per sample row x[256], scalar t):
  pi    = softmax(relu(x@W1p+b1p) @ W2p + b2p)                      [K=8]
  g     = x @ W1o[:-1] + b1o                                        [H=512]
  h_n   = relu(g + c_n * (t * w))     c_n=(1+u_n)/2, w=W1o[-1]      [NQ, 512]
  f_n   = softplus(h_n @ W2o + b2o)                                 [NQ, 8]
  pred  = (t/2) * sum_n W_n f_n                                     [8]
  preds = pi * (1 - exp(-pred))
returns (preds, pi)

Implementation notes:
- NQ quadrature nodes (reference uses 15; Gauss-Legendre converges so fast on
  this integrand that NQ=6 matches the 15-node reference to ~7e-5, far inside
  the 2e-2 gate; bf16 rounding dominates the error at ~4e-3).
- bf16 operands everywhere on the PE; fp32 PSUM accumulation.
- layer-2 packs up to 4 quadrature nodes (and the pi-head logits) into one
  PSUM tile via column tile_position strips -> 4 concurrent matmuls, and the
  softplus + quadrature sum then run at 104-partition width instead of 8.
- softplus is a single ACT pass (softplus_and_others table also has relu).
- quadrature sum is a PE matmul against a strip-weight matrix R with
  -W_n/2 folded in; the (t) factor is applied during the psum evacuation.
- h build: mix of DVE route (stt: t_bcast*w_pc + g, then relu) and PE route
  (identity re-inject + rank-1 into psum, relu-evac on ACT/DVE).
"""

import os
import sys

for _p in (
    "/root/.axon_site",
    "/root/.axon_site/_ro/trn_rl_repo",
    "/root/.axon_site/_ro/pypackages",
    "/opt/trn_rl_repo",
):
    if os.path.isdir(_p) and _p not in sys.path:
        sys.path.append(_p)

import numpy as np

import concourse.bass as bass
import concourse.mybir as mybir
import concourse.tile as tile
from concourse import bacc
from concourse.bass_utils import run_bass_kernel_spmd
from concourse.masks import make_identity

F32 = mybir.dt.float32
BF = mybir.dt.bfloat16
AX = mybir.AxisListType
OP = mybir.AluOpType
AF = mybir.ActivationFunctionType

# Steer the greedy act-table selector: keep set ORDER identical (the emitted
# act_func_set_id is a positional index), but hide Exp/Ln/Relu/Copy/Identity
# from all other sets so the whole kernel uses the one combined set (1 load).
_orig_get_tables = bacc.get_activation_tables


def _tables_lnexp_first(arch):
    t = _orig_get_tables(arch)
    pref = "natural_log_exp_and_others"
    if pref not in t:
        return t
    hide = {AF.Exp, AF.Ln, AF.Relu, AF.Copy, AF.Identity}
    out = {}
    for k, v in t.items():
        if k != pref and (v & hide):
            v = v - hide
        out[k] = v
    return out


bacc.get_activation_tables = _tables_lnexp_first

N_CORES = 8
B_FULL, COV, H, K = 16384, 256, 512, 8
B = B_FULL // N_CORES  # 2048 per core
TT, TS = 4, 512        # batch column tiles
C = H // 128           # 4 H-chunks
CIN = COV // 128       # 2 cov-chunks

# --- tuning knobs -----------------------------------------------------------
NQ = 5                 # quadrature nodes (ref=15; 5 -> 1e-4 quad error)
PE_ROUTE = 6           # of NQ*C h units, how many go PE+evac (rest DVE stt)
PE_EVAC_DVE = 0        # of the PE-route units, how many evac on DVE (rest ACT)
RELU_ACT = 0           # of the DVE-route units, how many relu on ACT
RELU_GPS = 0           # of the DVE-route units, how many relu on GPSIMD (slow!)
G_EVAC_ACT = 8         # of the 16 g evacs, how many on ACT (rest DVE)
H1P_EVAC_DVE = 0       # of the 16 h1p evacs, how many on DVE (rest ACT)
# ---------------------------------------------------------------------------

_u64, _w64 = np.polynomial.legendre.leggauss(NQ)
CN = [float(np.float32(0.5) * (np.float32(1.0) + u)) for u in _u64.astype(np.float32)]
WN = [float(w) for w in _w64.astype(np.float32)]

# layer-2 strip groups: chunks of up to 4 nodes; pi head rides in the last
# group's strip 3 (or its own group if the last one is full).
_node_groups = [list(range(i, min(i + 4, NQ))) for i in range(0, NQ, 4)]
if len(_node_groups[-1]) <= 3:
    PI_GROUP = len(_node_groups) - 1
else:
    _node_groups.append([])
    PI_GROUP = len(_node_groups) - 1
PI_STRIP = 3
NGROUPS = len(_node_groups)


def _spread(n_total, count):
    return {i for i in range(n_total) if ((i + 1) * count) // n_total > (i * count) // n_total}


def build_kernel():
    nc = bacc.Bacc("TRN2", target_bir_lowering=False, debug=False)

    x_d = nc.dram_tensor("x", [B, COV], F32, kind="ExternalInput").ap()
    t_d = nc.dram_tensor("t", [B], F32, kind="ExternalInput").ap()
    w1p_d = nc.dram_tensor("W1p", [COV, H], F32, kind="ExternalInput").ap()
    b1p_d = nc.dram_tensor("b1p", [H], F32, kind="ExternalInput").ap()
    w2p_d = nc.dram_tensor("W2p", [H, K], F32, kind="ExternalInput").ap()
    b2p_d = nc.dram_tensor("b2p", [K], F32, kind="ExternalInput").ap()
    w1o_d = nc.dram_tensor("W1o", [COV + 1, H], F32, kind="ExternalInput").ap()
    b1o_d = nc.dram_tensor("b1o", [H], F32, kind="ExternalInput").ap()
    w2o_d = nc.dram_tensor("W2o", [H, K], F32, kind="ExternalInput").ap()
    b2o_d = nc.dram_tensor("b2o", [K], F32, kind="ExternalInput").ap()
    preds_d = nc.dram_tensor("preds", [B, K], F32, kind="ExternalOutput").ap()
    pi_d = nc.dram_tensor("pi", [B, K], F32, kind="ExternalOutput").ap()

    n_units = NQ * C
    pe_units = _spread(n_units, PE_ROUTE)
    pe_units_l = sorted(pe_units)
    pe_evac_dve = {pe_units_l[i] for i in sorted(_spread(len(pe_units_l), PE_EVAC_DVE))} if pe_units_l else set()
    dve_units_l = sorted(set(range(n_units)) - pe_units)
    relu_act = {dve_units_l[i] for i in sorted(_spread(len(dve_units_l), RELU_ACT))} if dve_units_l else set()
    rest_l = [i for i in dve_units_l if i not in relu_act]
    relu_gps = {rest_l[i] for i in sorted(_spread(len(rest_l), RELU_GPS))} if rest_l else set()
    g_evac_act = _spread(16, G_EVAC_ACT)
    h1p_evac_dve = _spread(16, H1P_EVAC_DVE)

    with tile.TileContext(nc) as tc:
        with (
            tc.tile_pool(name="pers", bufs=1) as pers,
            tc.tile_pool(name="ph", bufs=n_units) as ph,
            tc.tile_pool(name="pxin", bufs=2) as pxin,
            tc.tile_pool(name="pft", bufs=4) as pft,
            tc.tile_pool(name="psm", bufs=1) as psm,
            tc.tile_pool(name="pps", bufs=4, space="PSUM") as pps,
            tc.tile_pool(name="ppsf", bufs=2, space="PSUM") as ppsf,
            tc.tile_pool(name="ppred", bufs=2, space="PSUM") as ppred,
        ):
            def pt(name, shape, dt=F32):
                return pers.tile(shape, dt, tag=name, name=name)

            # ---- persistent SBUF tiles ----
            ident128 = pt("ident128", [128, 128])          # fp32, for x transpose
            identB = pt("identB", [128, 128], BF)          # bf16, for PE h route
            ident8 = pt("ident8", [8, 8])                  # fp32, small transposes
            xT = pt("xT", [128, CIN * B], BF)              # [128, ci*2048+b]
            g_sb = [pt(f"g{c}", [128, B], BF) for c in range(C)]
            h1p_sb = [pt(f"h1p{c}", [128, B], BF) for c in range(C)]
            t_bcast = pt("t_bcast", [128, B], BF)
            t_row_bf = pt("t_row_bf", [1, B], BF)
            ones_row = pt("ones_row", [1, 128], BF)
            w1o_sb = [pt(f"w1o{ci}", [128, H], BF) for ci in range(CIN)]
            w1p_sb = [pt(f"w1p{ci}", [128, H], BF) for ci in range(CIN)]
            w2o_sb = [pt(f"w2o{c}", [128, K], BF) for c in range(C)]
            w2p_sb = [pt(f"w2p{c}", [128, K], BF) for c in range(C)]
            w_row = pt("w_row", [1, H])                    # fp32 W1o[-1]
            w_pc = pt("w_pc", [128, C])                    # fp32 W1o[-1] as [p,c]
            wsc_row = [pt(f"wscr{n}", [1, H], BF) for n in range(NQ)]
            wsc_pc = [pt(f"wscp{n}", [128, C], BF) for n in range(NQ)]
            b1o_pc = pt("b1o_pc", [128, C])
            b1p_pc = pt("b1p_pc", [128, C])
            b2o_col = pt("b2o_col", [128, 1])
            b2p_col = pt("b2p_col", [128, 1])
            R_sb = [pt(f"R{g}", [128, 8], BF) for g in range(NGROUPS)]
            pred_sb = pt("pred_sb", [8, B])
            lgt_sb = pt("lgt_sb", [8, B])
            pred_b = pt("pred_b", [128, B // 128 * K])
            logits_b = pt("logits_b", [128, B // 128 * K])
            e_b = pt("e_b", [128, B // 128 * K])
            eneg = pt("eneg", [128, B // 128 * K])
            sums = pt("sums", [128, B // 128])
            rec = pt("rec", [128, B // 128])
            pi_b = pt("pi_b", [128, B // 128 * K])
            preds_b = pt("preds_b", [128, B // 128 * K])

            # ---- constants ----
            make_identity(nc, ident128)
            make_identity(nc, identB)
            make_identity(nc, ident8)
            nc.vector.memset(ones_row, 1.0)
            nc.vector.memset(b2o_col, 0.0)
            nc.vector.memset(b2p_col, 0.0)

            # ---- weight / small input DMAs + bf16 casts ----
            # consolidate into few big transfers: one dma_start spreads across
            # all 16 SDMA engines, and each trigger costs ~600ns of queue time
            w1o_ld = psm.tile([128, CIN * H], F32, tag="wld", name="w1old")
            nc.sync.dma_start(
                out=w1o_ld.rearrange("p (ci q) -> p ci q", ci=CIN),
                in_=w1o_d[0:COV, :].rearrange("(ci p) q -> p ci q", p=128),
            )
            for ci in range(CIN):
                nc.vector.tensor_copy(w1o_sb[ci], w1o_ld[:, ci * H : (ci + 1) * H])
            w1p_ld = psm.tile([128, CIN * H], F32, tag="wld", name="w1pld")
            nc.sync.dma_start(
                out=w1p_ld.rearrange("p (ci q) -> p ci q", ci=CIN),
                in_=w1p_d.rearrange("(ci p) q -> p ci q", p=128),
            )
            for ci in range(CIN):
                nc.vector.tensor_copy(w1p_sb[ci], w1p_ld[:, ci * H : (ci + 1) * H])
            w2o_ld = psm.tile([128, C * K], F32, tag="w2ld", name="w2old")
            nc.scalar.dma_start(
                out=w2o_ld.rearrange("p (c k) -> p c k", c=C),
                in_=w2o_d.rearrange("(c p) k -> p c k", p=128),
            )
            w2p_ld = psm.tile([128, C * K], F32, tag="w2ld2", name="w2pld")
            nc.scalar.dma_start(
                out=w2p_ld.rearrange("p (c k) -> p c k", c=C),
                in_=w2p_d.rearrange("(c p) k -> p c k", p=128),
            )
            for c in range(C):
                nc.vector.tensor_copy(w2o_sb[c], w2o_ld[:, c * K : (c + 1) * K])
                nc.vector.tensor_copy(w2p_sb[c], w2p_ld[:, c * K : (c + 1) * K])

            t_row_ld = pers.tile([1, B], F32, tag="trow", name="t_row_ld")
            nc.scalar.dma_start(out=t_row_ld, in_=t_d.rearrange("(a b) -> a b", a=1))
            nc.vector.tensor_copy(t_row_bf, t_row_ld)
            nc.scalar.dma_start(out=w_row, in_=w1o_d[COV : COV + 1, :])
            nc.scalar.dma_start(
                out=w_pc, in_=w1o_d[COV : COV + 1, :].rearrange("a (c p) -> p (c a)", p=128)
            )
            for n in range(NQ):
                nc.vector.tensor_scalar_mul(wsc_row[n], w_row, CN[n])
                nc.vector.tensor_scalar_mul(wsc_pc[n], w_pc, CN[n])
            nc.scalar.dma_start(out=b1o_pc, in_=b1o_d.rearrange("(c p) -> p c", p=128))
            nc.scalar.dma_start(out=b1p_pc, in_=b1p_d.rearrange("(c p) -> p c", p=128))
            for j in range(4):
                nc.scalar.dma_start(
                    out=b2o_col[32 * j : 32 * j + 8, :],
                    in_=b2o_d.rearrange("(k a) -> k a", a=1),
                )
            nc.scalar.dma_start(
                out=b2p_col[32 * PI_STRIP : 32 * PI_STRIP + 8, :],
                in_=b2p_d.rearrange("(k a) -> k a", a=1),
            )
            # strip-weight matrices: R[g][32j+k, k] = -0.5 * WN[node], else 0
            for g, nodes in enumerate(_node_groups):
                nc.vector.memset(R_sb[g], 0.0)
                for j, n in enumerate(nodes):
                    nc.scalar.activation(
                        R_sb[g][32 * j : 32 * j + 8, :], ident8, AF.Copy,
                        scale=-0.5 * WN[n],
                    )

            # ---- x load + transpose + cast to bf16 (feature-major xT) ----
            # x comes in 4 big DMAs of 512 rows each ([128, 4, 256] row-
            # interleaved); 4 transposes batch into one psum bank and one ACT
            # copy evacuates them (cast to bf16) via a 4D AP.
            xT_v = xT.rearrange("p (ci b) -> p ci b", ci=CIN)
            for blk in range(4):
                xin = pxin.tile([128, 4 * COV], F32, tag="xin", name=f"xin_{blk}")
                dma_eng = nc.sync if blk % 2 == 0 else nc.scalar
                dma_eng.dma_start(
                    out=xin.rearrange("p (q c) -> p q c", q=4),
                    in_=x_d[blk * 512 : (blk + 1) * 512, :].rearrange(
                        "(q p) c -> p q c", p=128
                    ),
                )
                for half in range(2):
                    pxt = pps.tile([128, 512], F32, tag="ps", name=f"pxt_{blk}_{half}")
                    for jj in range(2):
                        q = half * 2 + jj
                        for ci in range(CIN):
                            nc.tensor.transpose(
                                pxt[:, (jj * 2 + ci) * 128 : (jj * 2 + ci + 1) * 128],
                                xin[:, q * COV + ci * 128 : q * COV + (ci + 1) * 128],
                                ident128,
                            )
                    r0 = blk * 512 + half * 256
                    evac_eng = nc.scalar if (blk * 2 + half) % 2 == 0 else nc.vector
                    if evac_eng is nc.scalar:
                        nc.scalar.copy(
                            xT_v[:, :, r0 : r0 + 256].rearrange(
                                "p ci (jj q) -> p jj ci q", jj=2
                            ),
                            pxt.rearrange("p (jj ci q) -> p jj ci q", jj=2, ci=CIN),
                        )
                    else:
                        nc.vector.tensor_copy(
                            xT_v[:, :, r0 : r0 + 256].rearrange(
                                "p ci (jj q) -> p jj ci q", jj=2
                            ),
                            pxt.rearrange("p (jj ci q) -> p jj ci q", jj=2, ci=CIN),
                        )

            # ---- t_bcast[p, b] = t[b] (rank-1 ones x t) ----
            for T in range(TT):
                bs = slice(T * TS, (T + 1) * TS)
                pst = pps.tile([128, TS], F32, tag="ps", name=f"ptb_{T}")
                nc.tensor.matmul(pst, ones_row, t_row_bf[:, bs], start=True, stop=True)
                nc.vector.tensor_copy(t_bcast[:, bs], pst)

            # ---- layer-1 matmuls (both nets) ----
            for c in range(C):
                cs = slice(c * 128, (c + 1) * 128)
                for T in range(TT):
                    bs = slice(T * TS, (T + 1) * TS)
                    i = c * TT + T
                    pso = pps.tile([128, TS], F32, tag="ps", name=f"pso_{c}_{T}")
                    for ci in range(CIN):
                        nc.tensor.matmul(
                            pso, w1o_sb[ci][:, cs],
                            xT_v[:, ci, T * TS : (T + 1) * TS],
                            start=(ci == 0), stop=(ci == CIN - 1),
                        )
                    if i in g_evac_act:
                        nc.scalar.activation(
                            g_sb[c][:, bs], pso, AF.Identity, bias=b1o_pc[:, c : c + 1]
                        )
                    else:
                        nc.vector.tensor_scalar_add(
                            g_sb[c][:, bs], pso, b1o_pc[:, c : c + 1]
                        )
                    psp = pps.tile([128, TS], F32, tag="ps", name=f"psp_{c}_{T}")
                    for ci in range(CIN):
                        nc.tensor.matmul(
                            psp, w1p_sb[ci][:, cs],
                            xT_v[:, ci, T * TS : (T + 1) * TS],
                            start=(ci == 0), stop=(ci == CIN - 1),
                        )
                    if i in h1p_evac_dve:
                        nc.vector.tensor_scalar(
                            h1p_sb[c][:, bs], psp, b1p_pc[:, c : c + 1], 0.0,
                            OP.add, OP.max,
                        )
                    else:
                        nc.scalar.activation(
                            h1p_sb[c][:, bs], psp, AF.Relu, bias=b1p_pc[:, c : c + 1]
                        )

            # ---- G2[c] = t_bcast * w_pc[:, c] (bf16, feeds the TT h route) ----
            G2 = [pt(f"G2_{c}", [128, B], BF) for c in range(C)]
            for c in range(C):
                for half in range(2):
                    hb = slice(half * (B // 2), (half + 1) * (B // 2))
                    nc.vector.tensor_scalar_mul(
                        G2[c][:, hb], t_bcast[:, hb], w_pc[:, c : c + 1]
                    )

            # ---- h units (half-B granularity so layer-2 pipelines earlier) ----
            h_tiles = {}
            for n in range(NQ):
                for c in range(C):
                    h_tiles[(n, c)] = ph.tile([128, B], BF, tag="h", name=f"h_{n}_{c}")

            def emit_h_units(half):
                hb = slice(half * (B // 2), (half + 1) * (B // 2))
                for n in range(NQ):
                    for c in range(C):
                        i = n * C + c
                        ht = h_tiles[(n, c)]
                        if i in pe_units:
                            cs = slice(c * 128, (c + 1) * 128)
                            pshs = []
                            for Th in range(2):
                                T = half * 2 + Th
                                bs = slice(T * TS, (T + 1) * TS)
                                psh = pps.tile(
                                    [128, TS], F32, tag="ps", name=f"psh_{n}_{c}_{T}"
                                )
                                nc.tensor.matmul(
                                    psh, identB, g_sb[c][:, bs], start=True, stop=False
                                )
                                pshs.append(psh)
                            for Th in range(2):
                                T = half * 2 + Th
                                bs = slice(T * TS, (T + 1) * TS)
                                nc.tensor.matmul(
                                    pshs[Th], wsc_row[n][:, cs], t_row_bf[:, bs],
                                    start=False, stop=True,
                                )
                            for Th in range(2):
                                T = half * 2 + Th
                                bs = slice(T * TS, (T + 1) * TS)
                                if i in pe_evac_dve:
                                    nc.vector.tensor_scalar_max(ht[:, bs], pshs[Th], 0.0)
                                else:
                                    nc.scalar.activation(ht[:, bs], pshs[Th], AF.Relu)
                        else:
                            # ht = relu(c_n * G2 + g): TS-mul (4x) + TT-add (2x)
                            # + TS-max (4x) all in bf16 fast modes
                            nc.vector.tensor_scalar_mul(ht[:, hb], G2[c][:, hb], CN[n])
                            nc.vector.tensor_tensor(
                                out=ht[:, hb], in0=ht[:, hb], in1=g_sb[c][:, hb],
                                op=OP.add,
                            )
                            if i in relu_act:
                                nc.scalar.activation(ht[:, hb], ht[:, hb], AF.Relu)
                            else:
                                nc.vector.tensor_scalar_max(ht[:, hb], ht[:, hb], 0.0)

            # ---- main pipeline: h units (per half), then per-T layer-2
            #      (col-tiled strips) + softplus + quadrature + final chain ----
            fgroups = [g for g, nodes in enumerate(_node_groups) if nodes]
            psf_count = 0
            for half in range(2):
                emit_h_units(half)
                for Th in range(2):
                    T = half * 2 + Th
                    bs = slice(T * TS, (T + 1) * TS)
                    f_tiles = {}
                    for g, nodes in enumerate(_node_groups):
                        psf = ppsf.tile([128, TS], F32, tag="psf", name=f"psf_{g}_{T}")
                        if psf_count < 2:
                            # first touch of each ring slot: clear garbage rows
                            # so exp of unwritten partitions stays finite
                            nc.vector.memset(psf, 0.0)
                        psf_count += 1
                        for c in range(C):
                            for j, n in enumerate(nodes):
                                nc.tensor.matmul(
                                    psf[32 * j : 32 * j + 8, :],
                                    w2o_sb[c], h_tiles[(n, c)][:, bs],
                                    start=(c == 0), stop=(c == C - 1),
                                    tile_position=(0, 32 * j),
                                )
                            if g == PI_GROUP:
                                nc.tensor.matmul(
                                    psf[32 * PI_STRIP : 32 * PI_STRIP + 8, :],
                                    w2p_sb[c], h1p_sb[c][:, bs],
                                    start=(c == 0), stop=(c == C - 1),
                                    tile_position=(0, 32 * PI_STRIP),
                                )
                        if nodes:
                            top = 32 * (len(nodes) - 1) + 8
                            et = pft.tile([top, TS], BF, tag="et", name=f"et_{g}_{T}")
                            nc.scalar.activation(
                                et, psf[0:top, :], AF.Exp, bias=b2o_col[0:top, :]
                            )
                            ft = pft.tile([top, TS], BF, tag="ft", name=f"ft_{g}_{T}")
                            nc.scalar.activation(ft, et, AF.Ln, bias=1.0)
                            f_tiles[g] = ft
                        if g == PI_GROUP:
                            nc.vector.tensor_scalar_add(
                                lgt_sb[:, bs],
                                psf[32 * PI_STRIP : 32 * PI_STRIP + 8, :],
                                b2p_col[32 * PI_STRIP : 32 * PI_STRIP + 8, :],
                            )

                    # quadrature: pred_sb = -(t/2) sum_n W_n f_n
                    pp = ppred.tile([8, TS], F32, tag="ppred", name=f"ppred_{T}")
                    for k, g in enumerate(fgroups):
                        top = 32 * (len(_node_groups[g]) - 1) + 8
                        nc.tensor.matmul(
                            pp, R_sb[g][0:top, :], f_tiles[g],
                            start=(k == 0), stop=(k == len(fgroups) - 1),
                        )
                    # fold in the per-column t factor during evacuation
                    nc.vector.scalar_tensor_tensor(
                        out=pred_sb[:, bs], in0=pp, scalar=1.0,
                        in1=t_bcast[0:8, bs], op0=OP.mult, op1=OP.mult,
                    )

                    # transpose pred + logits to batch-major
                    pxp = pps.tile([128, 32], F32, tag="ps", name=f"pxp_{T}")
                    pxl = pps.tile([128, 32], F32, tag="ps", name=f"pxl_{T}")
                    for j in range(4):
                        off = T * TS + j * 128
                        nc.tensor.transpose(
                            pxp[:, j * 8 : (j + 1) * 8], pred_sb[:, off : off + 128],
                            ident8,
                        )
                        nc.tensor.transpose(
                            pxl[:, j * 8 : (j + 1) * 8], lgt_sb[:, off : off + 128],
                            ident8,
                        )
                    ts32 = slice(T * 32, (T + 1) * 32)
                    nc.vector.tensor_copy(pred_b[:, ts32], pxp)
                    nc.vector.tensor_copy(logits_b[:, ts32], pxl)

                    # per-T final: softmax(pi), cif, preds
                    nc.scalar.activation(e_b[:, ts32], logits_b[:, ts32], AF.Exp)
                    nc.scalar.activation(eneg[:, ts32], pred_b[:, ts32], AF.Exp)
                    nc.vector.tensor_reduce(
                        sums[:, T * 4 : (T + 1) * 4],
                        e_b[:, ts32].rearrange("p (t k) -> p t k", k=8),
                        axis=AX.X, op=OP.add,
                    )
                    nc.vector.reciprocal(
                        rec[:, T * 4 : (T + 1) * 4], sums[:, T * 4 : (T + 1) * 4]
                    )
                    for jj in range(T * 4, (T + 1) * 4):
                        nc.vector.tensor_scalar_mul(
                            pi_b[:, jj * 8 : (jj + 1) * 8],
                            e_b[:, jj * 8 : (jj + 1) * 8],
                            rec[:, jj : jj + 1],
                        )
                    nc.vector.tensor_scalar(
                        eneg[:, ts32], eneg[:, ts32], -1.0, 1.0, OP.mult, OP.add
                    )
                    nc.vector.tensor_tensor(
                        out=preds_b[:, ts32], in0=eneg[:, ts32], in1=pi_b[:, ts32],
                        op=OP.mult,
                    )

            nc.sync.dma_start(
                out=preds_d.rearrange("(j p) k -> p j k", p=128),
                in_=preds_b.rearrange("p (j k) -> p j k", k=8),
            )
            nc.sync.dma_start(
                out=pi_d.rearrange("(j p) k -> p j k", p=128),
                in_=pi_b.rearrange("p (j k) -> p j k", k=8),
            )

    nc.compile()
    return nc


_NC = None


def _get_nc():
    global _NC
    if _NC is None:
        _NC = build_kernel()
    return _NC


def _shard_inputs(inputs):
    in_maps = []
    for i in range(N_CORES):
        sl = slice(i * B, (i + 1) * B)
        m = {
            "x": np.ascontiguousarray(np.asarray(inputs["x"], np.float32)[sl]),
            "t": np.ascontiguousarray(np.asarray(inputs["t"], np.float32)[sl]),
        }
        for k in ("W1p", "b1p", "W2p", "b2p", "W1o", "b1o", "W2o", "b2o"):
            m[k] = np.asarray(inputs[k], np.float32)
        in_maps.append(m)
    return in_maps


def kernel(**inputs):
    nc = _get_nc()
    in_maps = _shard_inputs(inputs)
    res = run_bass_kernel_spmd(nc, in_maps, core_ids=list(range(N_CORES)))
    preds = np.concatenate([res.results[i]["preds"] for i in range(N_CORES)], axis=0)
    pi = np.concatenate([res.results[i]["pi"] for i in range(N_CORES)], axis=0)
    return (preds, pi)


# revision 28
# speedup vs baseline: 2.1096x; 1.0361x over previous
"""Trainium2 Bass kernel for nn_ODESurvMultiple (dense_mlp, 8-core data parallel).

reference math (per sample row x[256], scalar t):
  pi    = softmax(relu(x@W1p+b1p) @ W2p + b2p)                      [K=8]
  g     = x @ W1o[:-1] + b1o                                        [H=512]
  h_n   = relu(g + c_n * (t * w))     c_n=(1+u_n)/2, w=W1o[-1]      [NQ, 512]
  f_n   = softplus(h_n @ W2o + b2o)                                 [NQ, 8]
  pred  = (t/2) * sum_n W_n f_n                                     [8]
  preds = pi * (1 - exp(-pred))
returns (preds, pi)

Implementation notes:
- NQ quadrature nodes (reference uses 15; Gauss-Legendre converges so fast on
  this integrand that NQ=6 matches the 15-node reference to ~7e-5, far inside
  the 2e-2 gate; bf16 rounding dominates the error at ~4e-3).
- bf16 operands everywhere on the PE; fp32 PSUM accumulation.
- layer-2 packs up to 4 quadrature nodes (and the pi-head logits) into one
  PSUM tile via column tile_position strips -> 4 concurrent matmuls, and the
  softplus + quadrature sum then run at 104-partition width instead of 8.
- softplus is a single ACT pass (softplus_and_others table also has relu).
- quadrature sum is a PE matmul against a strip-weight matrix R with
  -W_n/2 folded in; the (t) factor is applied during the psum evacuation.
- h build: mix of DVE route (stt: t_bcast*w_pc + g, then relu) and PE route
  (identity re-inject + rank-1 into psum, relu-evac on ACT/DVE).
"""

import os
import sys

for _p in (
    "/root/.axon_site",
    "/root/.axon_site/_ro/trn_rl_repo",
    "/root/.axon_site/_ro/pypackages",
    "/opt/trn_rl_repo",
):
    if os.path.isdir(_p) and _p not in sys.path:
        sys.path.append(_p)

import numpy as np

import concourse.bass as bass
import concourse.mybir as mybir
import concourse.tile as tile
from concourse import bacc
from concourse.bass_utils import run_bass_kernel_spmd
from concourse.masks import make_identity

F32 = mybir.dt.float32
BF = mybir.dt.bfloat16
AX = mybir.AxisListType
OP = mybir.AluOpType
AF = mybir.ActivationFunctionType

# Steer the greedy act-table selector: keep set ORDER identical (the emitted
# act_func_set_id is a positional index), but hide Exp/Ln/Relu/Copy/Identity
# from all other sets so the whole kernel uses the one combined set (1 load).
_orig_get_tables = bacc.get_activation_tables


def _tables_lnexp_first(arch):
    t = _orig_get_tables(arch)
    pref = "natural_log_exp_and_others"
    if pref not in t:
        return t
    hide = {AF.Exp, AF.Ln, AF.Relu, AF.Copy, AF.Identity}
    out = {}
    for k, v in t.items():
        if k != pref and (v & hide):
            v = v - hide
        out[k] = v
    return out


bacc.get_activation_tables = _tables_lnexp_first

N_CORES = 8
B_FULL, COV, H, K = 16384, 256, 512, 8
B = B_FULL // N_CORES  # 2048 per core
TT, TS = 4, 512        # batch column tiles
C = H // 128           # 4 H-chunks
CIN = COV // 128       # 2 cov-chunks

# --- tuning knobs -----------------------------------------------------------
NQ = 5                 # quadrature nodes (ref=15; 5 -> 1e-4 quad error)
PE_ROUTE = 6           # of NQ*C h units, how many go PE+evac (rest DVE stt)
PE_EVAC_DVE = 0        # of the PE-route units, how many evac on DVE (rest ACT)
RELU_ACT = 0           # of the DVE-route units, how many relu on ACT
RELU_GPS = 0           # of the DVE-route units, how many relu on GPSIMD (slow!)
G_EVAC_ACT = 8         # of the 16 g evacs, how many on ACT (rest DVE)
H1P_EVAC_DVE = 0       # of the 16 h1p evacs, how many on DVE (rest ACT)
# ---------------------------------------------------------------------------

_u64, _w64 = np.polynomial.legendre.leggauss(NQ)
CN = [float(np.float32(0.5) * (np.float32(1.0) + u)) for u in _u64.astype(np.float32)]
WN = [float(w) for w in _w64.astype(np.float32)]

# layer-2 strip groups: chunks of up to 4 nodes; pi head rides in the last
# group's strip 3 (or its own group if the last one is full).
_node_groups = [list(range(i, min(i + 4, NQ))) for i in range(0, NQ, 4)]
if len(_node_groups[-1]) <= 3:
    PI_GROUP = len(_node_groups) - 1
else:
    _node_groups.append([])
    PI_GROUP = len(_node_groups) - 1
PI_STRIP = 3
NGROUPS = len(_node_groups)


def _spread(n_total, count):
    return {i for i in range(n_total) if ((i + 1) * count) // n_total > (i * count) // n_total}


def build_kernel():
    nc = bacc.Bacc("TRN2", target_bir_lowering=False, debug=False)

    x_d = nc.dram_tensor("x", [B, COV], F32, kind="ExternalInput").ap()
    t_d = nc.dram_tensor("t", [B], F32, kind="ExternalInput").ap()
    w1p_d = nc.dram_tensor("W1p", [COV, H], F32, kind="ExternalInput").ap()
    b1p_d = nc.dram_tensor("b1p", [H], F32, kind="ExternalInput").ap()
    w2p_d = nc.dram_tensor("W2p", [H, K], F32, kind="ExternalInput").ap()
    b2p_d = nc.dram_tensor("b2p", [K], F32, kind="ExternalInput").ap()
    w1o_d = nc.dram_tensor("W1o", [COV + 1, H], F32, kind="ExternalInput").ap()
    b1o_d = nc.dram_tensor("b1o", [H], F32, kind="ExternalInput").ap()
    w2o_d = nc.dram_tensor("W2o", [H, K], F32, kind="ExternalInput").ap()
    b2o_d = nc.dram_tensor("b2o", [K], F32, kind="ExternalInput").ap()
    preds_d = nc.dram_tensor("preds", [B, K], F32, kind="ExternalOutput").ap()
    pi_d = nc.dram_tensor("pi", [B, K], F32, kind="ExternalOutput").ap()

    n_units = NQ * C
    pe_units = _spread(n_units, PE_ROUTE)
    pe_units_l = sorted(pe_units)
    pe_evac_dve = {pe_units_l[i] for i in sorted(_spread(len(pe_units_l), PE_EVAC_DVE))} if pe_units_l else set()
    dve_units_l = sorted(set(range(n_units)) - pe_units)
    relu_act = {dve_units_l[i] for i in sorted(_spread(len(dve_units_l), RELU_ACT))} if dve_units_l else set()
    rest_l = [i for i in dve_units_l if i not in relu_act]
    relu_gps = {rest_l[i] for i in sorted(_spread(len(rest_l), RELU_GPS))} if rest_l else set()
    g_evac_act = _spread(16, G_EVAC_ACT)
    h1p_evac_dve = _spread(16, H1P_EVAC_DVE)

    with tile.TileContext(nc) as tc:
        with (
            tc.tile_pool(name="pers", bufs=1) as pers,
            tc.tile_pool(name="ph", bufs=n_units) as ph,
            tc.tile_pool(name="pxin", bufs=2) as pxin,
            tc.tile_pool(name="pft", bufs=4) as pft,
            tc.tile_pool(name="psm", bufs=1) as psm,
            tc.tile_pool(name="pps", bufs=4, space="PSUM") as pps,
            tc.tile_pool(name="ppsf", bufs=2, space="PSUM") as ppsf,
            tc.tile_pool(name="ppred", bufs=2, space="PSUM") as ppred,
        ):
            def pt(name, shape, dt=F32):
                return pers.tile(shape, dt, tag=name, name=name)

            # ---- persistent SBUF tiles ----
            ident128 = pt("ident128", [128, 128])          # fp32, for x transpose
            identB = pt("identB", [128, 128], BF)          # bf16, for PE h route
            ident8 = pt("ident8", [8, 8])                  # fp32, small transposes
            ident8b = pt("ident8b", [8, 8], BF)            # bf16 twin
            xT = pt("xT", [128, CIN * B], BF)              # [128, ci*2048+b]
            g_sb = [pt(f"g{c}", [128, B], BF) for c in range(C)]
            h1p_sb = [pt(f"h1p{c}", [128, B], BF) for c in range(C)]
            t_bcast = pt("t_bcast", [128, B], BF)
            t_row_bf = pt("t_row_bf", [1, B], BF)
            ones_row = pt("ones_row", [1, 128], BF)
            w1o_sb = [pt(f"w1o{ci}", [128, H], BF) for ci in range(CIN)]
            w1p_sb = [pt(f"w1p{ci}", [128, H], BF) for ci in range(CIN)]
            w2o_sb = [pt(f"w2o{c}", [128, K], BF) for c in range(C)]
            w2p_sb = [pt(f"w2p{c}", [128, K], BF) for c in range(C)]
            w_row = pt("w_row", [1, H])                    # fp32 W1o[-1]
            w_pc = pt("w_pc", [128, C])                    # fp32 W1o[-1] as [p,c]
            wsc_row = [pt(f"wscr{n}", [1, H], BF) for n in range(NQ)]
            wsc_pc = [pt(f"wscp{n}", [128, C], BF) for n in range(NQ)]
            b1o_pc = pt("b1o_pc", [128, C])
            b1p_pc = pt("b1p_pc", [128, C])
            b2o_col = pt("b2o_col", [128, 1])
            b2p_col = pt("b2p_col", [128, 1])
            R_sb = [pt(f"R{g}", [128, 8], BF) for g in range(NGROUPS)]
            pred_sb = pt("pred_sb", [8, B])
            lgt_sb = pt("lgt_sb", [8, B], BF)
            pred_b = pt("pred_b", [128, B // 128 * K])
            logits_b = pt("logits_b", [128, B // 128 * K])
            e_b = pt("e_b", [128, B // 128 * K])
            eneg = pt("eneg", [128, B // 128 * K])
            sums = pt("sums", [128, B // 128])
            rec = pt("rec", [128, B // 128])
            pi_b = pt("pi_b", [128, B // 128 * K])
            preds_b = pt("preds_b", [128, B // 128 * K])

            # ---- constants ----
            make_identity(nc, ident128)
            make_identity(nc, identB)
            make_identity(nc, ident8)
            make_identity(nc, ident8b)
            nc.vector.memset(ones_row, 1.0)
            warm_ps = pps.tile([128, 128], F32, tag="ps", name="warm_ps")
            for _w in range(50):
                nc.tensor.matmul(warm_ps, identB, identB, start=True, stop=True)
            nc.vector.memset(b2o_col, 0.0)
            nc.vector.memset(b2p_col, 0.0)

            # ---- weight / small input DMAs + bf16 casts ----
            # consolidate into few big transfers: one dma_start spreads across
            # all 16 SDMA engines, and each trigger costs ~600ns of queue time
            w1o_ld = psm.tile([128, CIN * H], F32, tag="wld", name="w1old")
            nc.sync.dma_start(
                out=w1o_ld.rearrange("p (ci q) -> p ci q", ci=CIN),
                in_=w1o_d[0:COV, :].rearrange("(ci p) q -> p ci q", p=128),
            )
            for ci in range(CIN):
                nc.vector.tensor_copy(w1o_sb[ci], w1o_ld[:, ci * H : (ci + 1) * H])
            w1p_ld = psm.tile([128, CIN * H], F32, tag="wld2", name="w1pld")
            nc.sync.dma_start(
                out=w1p_ld.rearrange("p (ci q) -> p ci q", ci=CIN),
                in_=w1p_d.rearrange("(ci p) q -> p ci q", p=128),
            )
            for ci in range(CIN):
                nc.vector.tensor_copy(w1p_sb[ci], w1p_ld[:, ci * H : (ci + 1) * H])
            w2o_ld = psm.tile([128, C * K], F32, tag="w2ld", name="w2old")
            nc.scalar.dma_start(
                out=w2o_ld.rearrange("p (c k) -> p c k", c=C),
                in_=w2o_d.rearrange("(c p) k -> p c k", p=128),
            )
            w2p_ld = psm.tile([128, C * K], F32, tag="w2ld2", name="w2pld")
            nc.scalar.dma_start(
                out=w2p_ld.rearrange("p (c k) -> p c k", c=C),
                in_=w2p_d.rearrange("(c p) k -> p c k", p=128),
            )
            for c in range(C):
                nc.vector.tensor_copy(w2o_sb[c], w2o_ld[:, c * K : (c + 1) * K])
                nc.vector.tensor_copy(w2p_sb[c], w2p_ld[:, c * K : (c + 1) * K])

            t_row_ld = pers.tile([1, B], F32, tag="trow", name="t_row_ld")
            nc.scalar.dma_start(out=t_row_ld, in_=t_d.rearrange("(a b) -> a b", a=1))
            nc.vector.tensor_copy(t_row_bf, t_row_ld)
            nc.scalar.dma_start(out=w_row, in_=w1o_d[COV : COV + 1, :])
            nc.scalar.dma_start(
                out=w_pc, in_=w1o_d[COV : COV + 1, :].rearrange("a (c p) -> p (c a)", p=128)
            )
            for n in range(NQ):
                nc.vector.tensor_scalar_mul(wsc_row[n], w_row, CN[n])
                nc.vector.tensor_scalar_mul(wsc_pc[n], w_pc, CN[n])
            nc.scalar.dma_start(out=b1o_pc, in_=b1o_d.rearrange("(c p) -> p c", p=128))
            nc.scalar.dma_start(out=b1p_pc, in_=b1p_d.rearrange("(c p) -> p c", p=128))
            for j in range(4):
                nc.scalar.dma_start(
                    out=b2o_col[32 * j : 32 * j + 8, :],
                    in_=b2o_d.rearrange("(k a) -> k a", a=1),
                )
            nc.scalar.dma_start(
                out=b2p_col[32 * PI_STRIP : 32 * PI_STRIP + 8, :],
                in_=b2p_d.rearrange("(k a) -> k a", a=1),
            )
            # strip-weight matrices: R[g][32j+k, k] = -0.5 * WN[node], else 0
            for g, nodes in enumerate(_node_groups):
                nc.vector.memset(R_sb[g], 0.0)
                for j, n in enumerate(nodes):
                    nc.scalar.activation(
                        R_sb[g][32 * j : 32 * j + 8, :], ident8, AF.Copy,
                        scale=-0.5 * WN[n],
                    )

            # ---- x load + transpose + cast to bf16 (feature-major xT) ----
            # x comes in 4 big DMAs of 512 rows each ([128, 4, 256] row-
            # interleaved); 4 transposes batch into one psum bank and one ACT
            # copy evacuates them (cast to bf16) via a 4D AP.
            xT_v = xT.rearrange("p (ci b) -> p ci b", ci=CIN)
            for blk in range(4):
                xin = pxin.tile([128, 4 * COV], F32, tag="xin", name=f"xin_{blk}")
                dma_eng = nc.sync if blk % 2 == 0 else nc.scalar
                dma_eng.dma_start(
                    out=xin.rearrange("p (q c) -> p q c", q=4),
                    in_=x_d[blk * 512 : (blk + 1) * 512, :].rearrange(
                        "(q p) c -> p q c", p=128
                    ),
                )
                for half in range(2):
                    pxt = pps.tile([128, 512], F32, tag="ps", name=f"pxt_{blk}_{half}")
                    for jj in range(2):
                        q = half * 2 + jj
                        for ci in range(CIN):
                            nc.tensor.transpose(
                                pxt[:, (jj * 2 + ci) * 128 : (jj * 2 + ci + 1) * 128],
                                xin[:, q * COV + ci * 128 : q * COV + (ci + 1) * 128],
                                ident128,
                            )
                    r0 = blk * 512 + half * 256
                    evac_eng = nc.scalar if (blk * 2 + half) % 2 == 0 else nc.vector
                    if evac_eng is nc.scalar:
                        nc.scalar.copy(
                            xT_v[:, :, r0 : r0 + 256].rearrange(
                                "p ci (jj q) -> p jj ci q", jj=2
                            ),
                            pxt.rearrange("p (jj ci q) -> p jj ci q", jj=2, ci=CIN),
                        )
                    else:
                        nc.vector.tensor_copy(
                            xT_v[:, :, r0 : r0 + 256].rearrange(
                                "p ci (jj q) -> p jj ci q", jj=2
                            ),
                            pxt.rearrange("p (jj ci q) -> p jj ci q", jj=2, ci=CIN),
                        )

            # ---- t_bcast[p, b] = t[b] (rank-1 ones x t) ----
            for T in range(TT):
                bs = slice(T * TS, (T + 1) * TS)
                pst = pps.tile([128, TS], F32, tag="ps", name=f"ptb_{T}")
                nc.tensor.matmul(pst, ones_row, t_row_bf[:, bs], start=True, stop=True)
                nc.vector.tensor_copy(t_bcast[:, bs], pst)

            # ---- layer-1 matmuls (both nets) ----
            for c in range(C):
                cs = slice(c * 128, (c + 1) * 128)
                for T in range(TT):
                    bs = slice(T * TS, (T + 1) * TS)
                    i = c * TT + T
                    pso = pps.tile([128, TS], F32, tag="ps", name=f"pso_{c}_{T}")
                    for ci in range(CIN):
                        nc.tensor.matmul(
                            pso, w1o_sb[ci][:, cs],
                            xT_v[:, ci, T * TS : (T + 1) * TS],
                            start=(ci == 0), stop=(ci == CIN - 1),
                        )
                    if i in g_evac_act:
                        nc.scalar.activation(
                            g_sb[c][:, bs], pso, AF.Identity, bias=b1o_pc[:, c : c + 1]
                        )
                    else:
                        nc.vector.tensor_scalar_add(
                            g_sb[c][:, bs], pso, b1o_pc[:, c : c + 1]
                        )
                    psp = pps.tile([128, TS], F32, tag="ps", name=f"psp_{c}_{T}")
                    for ci in range(CIN):
                        nc.tensor.matmul(
                            psp, w1p_sb[ci][:, cs],
                            xT_v[:, ci, T * TS : (T + 1) * TS],
                            start=(ci == 0), stop=(ci == CIN - 1),
                        )
                    if i in h1p_evac_dve:
                        nc.vector.tensor_scalar(
                            h1p_sb[c][:, bs], psp, b1p_pc[:, c : c + 1], 0.0,
                            OP.add, OP.max,
                        )
                    else:
                        nc.scalar.activation(
                            h1p_sb[c][:, bs], psp, AF.Relu, bias=b1p_pc[:, c : c + 1]
                        )

            # ---- G2[c] = t_bcast * w_pc[:, c] (bf16, feeds the TT h route) ----
            G2 = [pt(f"G2_{c}", [128, B], BF) for c in range(C)]
            for c in range(C):
                for half in range(2):
                    hb = slice(half * (B // 2), (half + 1) * (B // 2))
                    nc.vector.tensor_scalar_mul(
                        G2[c][:, hb], t_bcast[:, hb], w_pc[:, c : c + 1]
                    )

            # ---- h units (half-B granularity so layer-2 pipelines earlier) ----
            h_tiles = {}
            for n in range(NQ):
                for c in range(C):
                    h_tiles[(n, c)] = ph.tile([128, B], BF, tag="h", name=f"h_{n}_{c}")

            def emit_h_units(half):
                hb = slice(half * (B // 2), (half + 1) * (B // 2))
                for n in range(NQ):
                    for c in range(C):
                        i = n * C + c
                        ht = h_tiles[(n, c)]
                        if i in pe_units:
                            cs = slice(c * 128, (c + 1) * 128)
                            pshs = []
                            for Th in range(2):
                                T = half * 2 + Th
                                bs = slice(T * TS, (T + 1) * TS)
                                psh = pps.tile(
                                    [128, TS], F32, tag="ps", name=f"psh_{n}_{c}_{T}"
                                )
                                nc.tensor.matmul(
                                    psh, identB, g_sb[c][:, bs], start=True, stop=False
                                )
                                pshs.append(psh)
                            for Th in range(2):
                                T = half * 2 + Th
                                bs = slice(T * TS, (T + 1) * TS)
                                nc.tensor.matmul(
                                    pshs[Th], wsc_row[n][:, cs], t_row_bf[:, bs],
                                    start=False, stop=True,
                                )
                            for Th in range(2):
                                T = half * 2 + Th
                                bs = slice(T * TS, (T + 1) * TS)
                                if i in pe_evac_dve:
                                    nc.vector.tensor_scalar_max(ht[:, bs], pshs[Th], 0.0)
                                else:
                                    nc.scalar.activation(ht[:, bs], pshs[Th], AF.Relu)
                        else:
                            # ht = relu(c_n * G2 + g): TS-mul (4x) + TT-add (2x)
                            # + TS-max (4x) all in bf16 fast modes
                            nc.vector.tensor_scalar_mul(ht[:, hb], G2[c][:, hb], CN[n])
                            nc.vector.tensor_tensor(
                                out=ht[:, hb], in0=ht[:, hb], in1=g_sb[c][:, hb],
                                op=OP.add,
                            )
                            if i in relu_act:
                                nc.scalar.activation(ht[:, hb], ht[:, hb], AF.Relu)
                            else:
                                nc.vector.tensor_scalar_max(ht[:, hb], ht[:, hb], 0.0)

            # ---- main pipeline: h units (per half), then per-T layer-2
            #      (col-tiled strips) + softplus + quadrature + final chain ----
            fgroups = [g for g, nodes in enumerate(_node_groups) if nodes]
            psf_count = 0
            for half in range(2):
                emit_h_units(half)
                for Th in range(2):
                    T = half * 2 + Th
                    bs = slice(T * TS, (T + 1) * TS)
                    f_tiles = {}
                    for g, nodes in enumerate(_node_groups):
                        psf = ppsf.tile([128, TS], F32, tag="psf", name=f"psf_{g}_{T}")
                        if psf_count < 2:
                            # first touch of each ring slot: clear garbage rows
                            # so exp of unwritten partitions stays finite
                            nc.vector.memset(psf, 0.0)
                        psf_count += 1
                        for c in range(C):
                            for j, n in enumerate(nodes):
                                nc.tensor.matmul(
                                    psf[32 * j : 32 * j + 8, :],
                                    w2o_sb[c], h_tiles[(n, c)][:, bs],
                                    start=(c == 0), stop=(c == C - 1),
                                    tile_position=(0, 32 * j),
                                )
                            if g == PI_GROUP:
                                nc.tensor.matmul(
                                    psf[32 * PI_STRIP : 32 * PI_STRIP + 8, :],
                                    w2p_sb[c], h1p_sb[c][:, bs],
                                    start=(c == 0), stop=(c == C - 1),
                                    tile_position=(0, 32 * PI_STRIP),
                                )
                        if nodes:
                            top = 32 * (len(nodes) - 1) + 8
                            et = pft.tile([top, TS], BF, tag="et", name=f"et_{g}_{T}")
                            nc.scalar.activation(
                                et, psf[0:top, :], AF.Exp, bias=b2o_col[0:top, :]
                            )
                            ft = pft.tile([top, TS], BF, tag="ft", name=f"ft_{g}_{T}")
                            nc.scalar.activation(ft, et, AF.Ln, bias=1.0)
                            f_tiles[g] = ft
                        if g == PI_GROUP:
                            nc.vector.tensor_scalar_add(
                                lgt_sb[:, bs],
                                psf[32 * PI_STRIP : 32 * PI_STRIP + 8, :],
                                b2p_col[32 * PI_STRIP : 32 * PI_STRIP + 8, :],
                            )

                    # quadrature: pred_sb = -(t/2) sum_n W_n f_n
                    pp = ppred.tile([8, TS], F32, tag="ppred", name=f"ppred_{T}")
                    for k, g in enumerate(fgroups):
                        top = 32 * (len(_node_groups[g]) - 1) + 8
                        nc.tensor.matmul(
                            pp, R_sb[g][0:top, :], f_tiles[g],
                            start=(k == 0), stop=(k == len(fgroups) - 1),
                        )
                    # fold in the per-column t factor during evacuation
                    nc.vector.scalar_tensor_tensor(
                        out=pred_sb[:, bs], in0=pp, scalar=1.0,
                        in1=t_bcast[0:8, bs], op0=OP.mult, op1=OP.mult,
                    )

                    # transpose pred + logits to batch-major
                    pxp = pps.tile([128, 32], F32, tag="ps", name=f"pxp_{T}")
                    pxl = pps.tile([128, 32], BF, tag="ps", name=f"pxl_{T}")
                    for j in range(4):
                        off = T * TS + j * 128
                        nc.tensor.transpose(
                            pxp[:, j * 8 : (j + 1) * 8], pred_sb[:, off : off + 128],
                            ident8,
                        )
                        nc.tensor.transpose(
                            pxl[:, j * 8 : (j + 1) * 8], lgt_sb[:, off : off + 128],
                            ident8b,
                        )
                    ts32 = slice(T * 32, (T + 1) * 32)
                    nc.vector.tensor_copy(pred_b[:, ts32], pxp)
                    nc.vector.tensor_copy(logits_b[:, ts32], pxl)

                    # per-T final: softmax(pi), cif, preds
                    nc.scalar.activation(e_b[:, ts32], logits_b[:, ts32], AF.Exp)
                    nc.scalar.activation(eneg[:, ts32], pred_b[:, ts32], AF.Exp)
                    nc.vector.tensor_reduce(
                        sums[:, T * 4 : (T + 1) * 4],
                        e_b[:, ts32].rearrange("p (t k) -> p t k", k=8),
                        axis=AX.X, op=OP.add,
                    )
                    nc.vector.reciprocal(
                        rec[:, T * 4 : (T + 1) * 4], sums[:, T * 4 : (T + 1) * 4]
                    )
                    for jj in range(T * 4, (T + 1) * 4):
                        nc.vector.tensor_scalar_mul(
                            pi_b[:, jj * 8 : (jj + 1) * 8],
                            e_b[:, jj * 8 : (jj + 1) * 8],
                            rec[:, jj : jj + 1],
                        )
                    nc.vector.tensor_scalar(
                        eneg[:, ts32], eneg[:, ts32], -1.0, 1.0, OP.mult, OP.add
                    )
                    nc.vector.tensor_tensor(
                        out=preds_b[:, ts32], in0=eneg[:, ts32], in1=pi_b[:, ts32],
                        op=OP.mult,
                    )

            nc.sync.dma_start(
                out=preds_d.rearrange("(j p) k -> p j k", p=128),
                in_=preds_b.rearrange("p (j k) -> p j k", k=8),
            )
            nc.sync.dma_start(
                out=pi_d.rearrange("(j p) k -> p j k", p=128),
                in_=pi_b.rearrange("p (j k) -> p j k", k=8),
            )

    nc.compile()
    return nc


_NC = None


def _get_nc():
    global _NC
    if _NC is None:
        _NC = build_kernel()
    return _NC


def _shard_inputs(inputs):
    in_maps = []
    for i in range(N_CORES):
        sl = slice(i * B, (i + 1) * B)
        m = {
            "x": np.ascontiguousarray(np.asarray(inputs["x"], np.float32)[sl]),
            "t": np.ascontiguousarray(np.asarray(inputs["t"], np.float32)[sl]),
        }
        for k in ("W1p", "b1p", "W2p", "b2p", "W1o", "b1o", "W2o", "b2o"):
            m[k] = np.asarray(inputs[k], np.float32)
        in_maps.append(m)
    return in_maps


def kernel(**inputs):
    nc = _get_nc()
    in_maps = _shard_inputs(inputs)
    res = run_bass_kernel_spmd(nc, in_maps, core_ids=list(range(N_CORES)))
    preds = np.concatenate([res.results[i]["preds"] for i in range(N_CORES)], axis=0)
    pi = np.concatenate([res.results[i]["pi"] for i in range(N_CORES)], axis=0)
    return (preds, pi)


# revision 29
# speedup vs baseline: 2.2679x; 1.0750x over previous
"""Trainium2 Bass kernel for nn_ODESurvMultiple (dense_mlp, 8-core data parallel).

reference math (per sample row x[256], scalar t):
  pi    = softmax(relu(x@W1p+b1p) @ W2p + b2p)                      [K=8]
  g     = x @ W1o[:-1] + b1o                                        [H=512]
  h_n   = relu(g + c_n * (t * w))     c_n=(1+u_n)/2, w=W1o[-1]      [NQ, 512]
  f_n   = softplus(h_n @ W2o + b2o)                                 [NQ, 8]
  pred  = (t/2) * sum_n W_n f_n                                     [8]
  preds = pi * (1 - exp(-pred))
returns (preds, pi)

Implementation notes:
- NQ quadrature nodes (reference uses 15; Gauss-Legendre converges so fast on
  this integrand that NQ=6 matches the 15-node reference to ~7e-5, far inside
  the 2e-2 gate; bf16 rounding dominates the error at ~4e-3).
- bf16 operands everywhere on the PE; fp32 PSUM accumulation.
- layer-2 packs up to 4 quadrature nodes (and the pi-head logits) into one
  PSUM tile via column tile_position strips -> 4 concurrent matmuls, and the
  softplus + quadrature sum then run at 104-partition width instead of 8.
- softplus is a single ACT pass (softplus_and_others table also has relu).
- quadrature sum is a PE matmul against a strip-weight matrix R with
  -W_n/2 folded in; the (t) factor is applied during the psum evacuation.
- h build: mix of DVE route (stt: t_bcast*w_pc + g, then relu) and PE route
  (identity re-inject + rank-1 into psum, relu-evac on ACT/DVE).
"""

import os
import sys

for _p in (
    "/root/.axon_site",
    "/root/.axon_site/_ro/trn_rl_repo",
    "/root/.axon_site/_ro/pypackages",
    "/opt/trn_rl_repo",
):
    if os.path.isdir(_p) and _p not in sys.path:
        sys.path.append(_p)

import numpy as np

import concourse.bass as bass
import concourse.mybir as mybir
import concourse.tile as tile
from concourse import bacc
from concourse.bass_utils import run_bass_kernel_spmd
from concourse.masks import make_identity

F32 = mybir.dt.float32
BF = mybir.dt.bfloat16
AX = mybir.AxisListType
OP = mybir.AluOpType
AF = mybir.ActivationFunctionType

# Steer the greedy act-table selector: keep set ORDER identical (the emitted
# act_func_set_id is a positional index), but hide Exp/Ln/Relu/Copy/Identity
# from all other sets so the whole kernel uses the one combined set (1 load).
_orig_get_tables = bacc.get_activation_tables


def _tables_lnexp_first(arch):
    t = _orig_get_tables(arch)
    pref = "natural_log_exp_and_others"
    if pref not in t:
        return t
    hide = {AF.Exp, AF.Ln, AF.Relu, AF.Copy, AF.Identity}
    out = {}
    for k, v in t.items():
        if k != pref and (v & hide):
            v = v - hide
        out[k] = v
    return out


bacc.get_activation_tables = _tables_lnexp_first

N_CORES = 8
B_FULL, COV, H, K = 16384, 256, 512, 8
B = B_FULL // N_CORES  # 2048 per core
TT, TS = 4, 512        # batch column tiles
C = H // 128           # 4 H-chunks
CIN = COV // 128       # 2 cov-chunks

# --- tuning knobs -----------------------------------------------------------
NQ = 4                 # quadrature nodes (ref=15; 4 -> 1.2e-4 quad error)
PE_ROUTE = 6           # of NQ*C h units, how many go PE+evac (rest DVE stt)
PE_EVAC_DVE = 0        # of the PE-route units, how many evac on DVE (rest ACT)
RELU_ACT = 0           # of the DVE-route units, how many relu on ACT
RELU_GPS = 0           # of the DVE-route units, how many relu on GPSIMD (slow!)
G_EVAC_ACT = 16        # of the 16 g evacs, how many on ACT (rest DVE)
H1P_EVAC_DVE = 0       # of the 16 h1p evacs, how many on DVE (rest ACT)
# ---------------------------------------------------------------------------

_u64, _w64 = np.polynomial.legendre.leggauss(NQ)
CN = [float(np.float32(0.5) * (np.float32(1.0) + u)) for u in _u64.astype(np.float32)]
WN = [float(w) for w in _w64.astype(np.float32)]

# layer-2 strip groups: chunks of up to 4 nodes; pi head rides in the last
# group's strip 3 (or its own group if the last one is full).
_node_groups = [list(range(i, min(i + 4, NQ))) for i in range(0, NQ, 4)]
if len(_node_groups[-1]) <= 3:
    PI_GROUP = len(_node_groups) - 1
else:
    _node_groups.append([])
    PI_GROUP = len(_node_groups) - 1
PI_STRIP = 3
NGROUPS = len(_node_groups)


def _spread(n_total, count):
    return {i for i in range(n_total) if ((i + 1) * count) // n_total > (i * count) // n_total}


def build_kernel():
    nc = bacc.Bacc("TRN2", target_bir_lowering=False, debug=False)

    x_d = nc.dram_tensor("x", [B, COV], F32, kind="ExternalInput").ap()
    t_d = nc.dram_tensor("t", [B], F32, kind="ExternalInput").ap()
    w1p_d = nc.dram_tensor("W1p", [COV, H], F32, kind="ExternalInput").ap()
    b1p_d = nc.dram_tensor("b1p", [H], F32, kind="ExternalInput").ap()
    w2p_d = nc.dram_tensor("W2p", [H, K], F32, kind="ExternalInput").ap()
    b2p_d = nc.dram_tensor("b2p", [K], F32, kind="ExternalInput").ap()
    w1o_d = nc.dram_tensor("W1o", [COV + 1, H], F32, kind="ExternalInput").ap()
    b1o_d = nc.dram_tensor("b1o", [H], F32, kind="ExternalInput").ap()
    w2o_d = nc.dram_tensor("W2o", [H, K], F32, kind="ExternalInput").ap()
    b2o_d = nc.dram_tensor("b2o", [K], F32, kind="ExternalInput").ap()
    preds_d = nc.dram_tensor("preds", [B, K], F32, kind="ExternalOutput").ap()
    pi_d = nc.dram_tensor("pi", [B, K], F32, kind="ExternalOutput").ap()

    n_units = NQ * C
    pe_units = _spread(n_units, PE_ROUTE)
    pe_units_l = sorted(pe_units)
    pe_evac_dve = {pe_units_l[i] for i in sorted(_spread(len(pe_units_l), PE_EVAC_DVE))} if pe_units_l else set()
    dve_units_l = sorted(set(range(n_units)) - pe_units)
    relu_act = {dve_units_l[i] for i in sorted(_spread(len(dve_units_l), RELU_ACT))} if dve_units_l else set()
    rest_l = [i for i in dve_units_l if i not in relu_act]
    relu_gps = {rest_l[i] for i in sorted(_spread(len(rest_l), RELU_GPS))} if rest_l else set()
    g_evac_act = _spread(16, G_EVAC_ACT)
    h1p_evac_dve = _spread(16, H1P_EVAC_DVE)

    with tile.TileContext(nc) as tc:
        with (
            tc.tile_pool(name="pers", bufs=1) as pers,
            tc.tile_pool(name="ph", bufs=n_units) as ph,
            tc.tile_pool(name="pxin", bufs=2) as pxin,
            tc.tile_pool(name="pft", bufs=4) as pft,
            tc.tile_pool(name="psm", bufs=1) as psm,
            tc.tile_pool(name="pps", bufs=4, space="PSUM") as pps,
            tc.tile_pool(name="ppsf", bufs=2, space="PSUM") as ppsf,
            tc.tile_pool(name="ppred", bufs=2, space="PSUM") as ppred,
        ):
            def pt(name, shape, dt=F32):
                return pers.tile(shape, dt, tag=name, name=name)

            # ---- persistent SBUF tiles ----
            ident128 = pt("ident128", [128, 128])          # fp32, for x transpose
            identB = pt("identB", [128, 128], BF)          # bf16, for PE h route
            ident8 = pt("ident8", [8, 8])                  # fp32, small transposes
            ident8b = pt("ident8b", [8, 8], BF)            # bf16 twin
            xT = pt("xT", [128, CIN * B], BF)              # [128, ci*2048+b]
            g_sb = [pt(f"g{c}", [128, B], BF) for c in range(C)]
            h1p_sb = [pt(f"h1p{c}", [128, B], BF) for c in range(C)]
            t_bcast = pt("t_bcast", [128, B], BF)
            t_row_bf = pt("t_row_bf", [1, B], BF)
            ones_row = pt("ones_row", [1, 128], BF)
            w1o_sb = [pt(f"w1o{ci}", [128, H], BF) for ci in range(CIN)]
            w1p_sb = [pt(f"w1p{ci}", [128, H], BF) for ci in range(CIN)]
            w2o_sb = [pt(f"w2o{c}", [128, K], BF) for c in range(C)]
            w2p_sb = [pt(f"w2p{c}", [128, K], BF) for c in range(C)]
            w_row = pt("w_row", [1, H])                    # fp32 W1o[-1]
            w_pc = pt("w_pc", [128, C])                    # fp32 W1o[-1] as [p,c]
            wsc_row = [pt(f"wscr{n}", [1, H], BF) for n in range(NQ)]
            wsc_pc = [pt(f"wscp{n}", [128, C], BF) for n in range(NQ)]
            b1o_pc = pt("b1o_pc", [128, C])
            b1p_pc = pt("b1p_pc", [128, C])
            b2o_col = pt("b2o_col", [128, 1])
            b2p_col = pt("b2p_col", [128, 1])
            R_sb = [pt(f"R{g}", [128, 8], BF) for g in range(NGROUPS)]
            pred_sb = pt("pred_sb", [8, B])
            lgt_sb = pt("lgt_sb", [8, B])
            pred_b = pt("pred_b", [128, B // 128 * K])
            logits_b = pt("logits_b", [128, B // 128 * K])
            e_b = pt("e_b", [128, B // 128 * K])
            eneg = pt("eneg", [128, B // 128 * K])
            sums = pt("sums", [128, B // 128])
            rec = pt("rec", [128, B // 128])
            pi_b = pt("pi_b", [128, B // 128 * K])
            preds_b = pt("preds_b", [128, B // 128 * K])

            # ---- constants ----
            make_identity(nc, ident128)
            make_identity(nc, identB)
            make_identity(nc, ident8)
            make_identity(nc, ident8b)
            nc.vector.memset(ones_row, 1.0)
            warm_ps = pps.tile([128, 128], F32, tag="ps", name="warm_ps")
            for _w in range(50):
                nc.tensor.matmul(warm_ps, identB, identB, start=True, stop=True)
            nc.vector.memset(b2o_col, 0.0)
            nc.vector.memset(b2p_col, 0.0)

            # ---- weight / small input DMAs + bf16 casts ----
            # consolidate into few big transfers: one dma_start spreads across
            # all 16 SDMA engines, and each trigger costs ~600ns of queue time
            w1o_ld = psm.tile([128, CIN * H], F32, tag="wld", name="w1old")
            nc.sync.dma_start(
                out=w1o_ld.rearrange("p (ci q) -> p ci q", ci=CIN),
                in_=w1o_d[0:COV, :].rearrange("(ci p) q -> p ci q", p=128),
            )
            for ci in range(CIN):
                nc.vector.tensor_copy(w1o_sb[ci], w1o_ld[:, ci * H : (ci + 1) * H])
            w1p_ld = psm.tile([128, CIN * H], F32, tag="wld2", name="w1pld")
            nc.sync.dma_start(
                out=w1p_ld.rearrange("p (ci q) -> p ci q", ci=CIN),
                in_=w1p_d.rearrange("(ci p) q -> p ci q", p=128),
            )
            for ci in range(CIN):
                nc.vector.tensor_copy(w1p_sb[ci], w1p_ld[:, ci * H : (ci + 1) * H])
            w2o_ld = psm.tile([128, C * K], F32, tag="w2ld", name="w2old")
            nc.scalar.dma_start(
                out=w2o_ld.rearrange("p (c k) -> p c k", c=C),
                in_=w2o_d.rearrange("(c p) k -> p c k", p=128),
            )
            w2p_ld = psm.tile([128, C * K], F32, tag="w2ld2", name="w2pld")
            nc.scalar.dma_start(
                out=w2p_ld.rearrange("p (c k) -> p c k", c=C),
                in_=w2p_d.rearrange("(c p) k -> p c k", p=128),
            )
            for c in range(C):
                nc.vector.tensor_copy(w2o_sb[c], w2o_ld[:, c * K : (c + 1) * K])
                nc.vector.tensor_copy(w2p_sb[c], w2p_ld[:, c * K : (c + 1) * K])

            t_row_ld = pers.tile([1, B], F32, tag="trow", name="t_row_ld")
            nc.scalar.dma_start(out=t_row_ld, in_=t_d.rearrange("(a b) -> a b", a=1))
            nc.vector.tensor_copy(t_row_bf, t_row_ld)
            nc.scalar.dma_start(out=w_row, in_=w1o_d[COV : COV + 1, :])
            nc.scalar.dma_start(
                out=w_pc, in_=w1o_d[COV : COV + 1, :].rearrange("a (c p) -> p (c a)", p=128)
            )
            for n in range(NQ):
                nc.vector.tensor_scalar_mul(wsc_row[n], w_row, CN[n])
                nc.vector.tensor_scalar_mul(wsc_pc[n], w_pc, CN[n])
            nc.scalar.dma_start(out=b1o_pc, in_=b1o_d.rearrange("(c p) -> p c", p=128))
            nc.scalar.dma_start(out=b1p_pc, in_=b1p_d.rearrange("(c p) -> p c", p=128))
            for j in range(4):
                nc.scalar.dma_start(
                    out=b2o_col[32 * j : 32 * j + 8, :],
                    in_=b2o_d.rearrange("(k a) -> k a", a=1),
                )
            nc.scalar.dma_start(
                out=b2p_col[32 * PI_STRIP : 32 * PI_STRIP + 8, :],
                in_=b2p_d.rearrange("(k a) -> k a", a=1),
            )
            # strip-weight matrices: R[g][32j+k, k] = -0.5 * WN[node], else 0
            for g, nodes in enumerate(_node_groups):
                nc.vector.memset(R_sb[g], 0.0)
                for j, n in enumerate(nodes):
                    nc.scalar.activation(
                        R_sb[g][32 * j : 32 * j + 8, :], ident8, AF.Copy,
                        scale=-0.5 * WN[n],
                    )

            # ---- x load + transpose + cast to bf16 (feature-major xT) ----
            # x comes in 4 big DMAs of 512 rows each ([128, 4, 256] row-
            # interleaved); 4 transposes batch into one psum bank and one ACT
            # copy evacuates them (cast to bf16) via a 4D AP.
            xT_v = xT.rearrange("p (ci b) -> p ci b", ci=CIN)
            for blk in range(4):
                xin = pxin.tile([128, 4 * COV], F32, tag="xin", name=f"xin_{blk}")
                dma_eng = nc.sync if blk % 2 == 0 else nc.scalar
                dma_eng.dma_start(
                    out=xin.rearrange("p (q c) -> p q c", q=4),
                    in_=x_d[blk * 512 : (blk + 1) * 512, :].rearrange(
                        "(q p) c -> p q c", p=128
                    ),
                )
                for half in range(2):
                    pxt = pps.tile([128, 512], F32, tag="ps", name=f"pxt_{blk}_{half}")
                    for jj in range(2):
                        q = half * 2 + jj
                        for ci in range(CIN):
                            nc.tensor.transpose(
                                pxt[:, (jj * 2 + ci) * 128 : (jj * 2 + ci + 1) * 128],
                                xin[:, q * COV + ci * 128 : q * COV + (ci + 1) * 128],
                                ident128,
                            )
                    r0 = blk * 512 + half * 256
                    evac_eng = nc.scalar if (blk * 2 + half) % 2 == 0 else nc.vector
                    if evac_eng is nc.scalar:
                        nc.scalar.copy(
                            xT_v[:, :, r0 : r0 + 256].rearrange(
                                "p ci (jj q) -> p jj ci q", jj=2
                            ),
                            pxt.rearrange("p (jj ci q) -> p jj ci q", jj=2, ci=CIN),
                        )
                    else:
                        nc.vector.tensor_copy(
                            xT_v[:, :, r0 : r0 + 256].rearrange(
                                "p ci (jj q) -> p jj ci q", jj=2
                            ),
                            pxt.rearrange("p (jj ci q) -> p jj ci q", jj=2, ci=CIN),
                        )

            # ---- t_bcast[p, b] = t[b] (rank-1 ones x t) ----
            for T in range(TT):
                bs = slice(T * TS, (T + 1) * TS)
                pst = pps.tile([128, TS], F32, tag="ps", name=f"ptb_{T}")
                nc.tensor.matmul(pst, ones_row, t_row_bf[:, bs], start=True, stop=True)
                nc.vector.tensor_copy(t_bcast[:, bs], pst)

            # ---- layer-1 matmuls (both nets) ----
            for c in range(C):
                cs = slice(c * 128, (c + 1) * 128)
                for T in range(TT):
                    bs = slice(T * TS, (T + 1) * TS)
                    i = c * TT + T
                    pso = pps.tile([128, TS], F32, tag="ps", name=f"pso_{c}_{T}")
                    for ci in range(CIN):
                        nc.tensor.matmul(
                            pso, w1o_sb[ci][:, cs],
                            xT_v[:, ci, T * TS : (T + 1) * TS],
                            start=(ci == 0), stop=(ci == CIN - 1),
                        )
                    if i in g_evac_act:
                        nc.scalar.activation(
                            g_sb[c][:, bs], pso, AF.Identity, bias=b1o_pc[:, c : c + 1]
                        )
                    else:
                        nc.vector.tensor_scalar_add(
                            g_sb[c][:, bs], pso, b1o_pc[:, c : c + 1]
                        )
                    psp = pps.tile([128, TS], F32, tag="ps", name=f"psp_{c}_{T}")
                    for ci in range(CIN):
                        nc.tensor.matmul(
                            psp, w1p_sb[ci][:, cs],
                            xT_v[:, ci, T * TS : (T + 1) * TS],
                            start=(ci == 0), stop=(ci == CIN - 1),
                        )
                    if i in h1p_evac_dve:
                        nc.vector.tensor_scalar(
                            h1p_sb[c][:, bs], psp, b1p_pc[:, c : c + 1], 0.0,
                            OP.add, OP.max,
                        )
                    else:
                        nc.scalar.activation(
                            h1p_sb[c][:, bs], psp, AF.Relu, bias=b1p_pc[:, c : c + 1]
                        )

            # ---- G2[c] = t_bcast * w_pc[:, c] (bf16, feeds the TT h route) ----
            G2 = [pt(f"G2_{c}", [128, B], BF) for c in range(C)]
            for c in range(C):
                for half in range(2):
                    hb = slice(half * (B // 2), (half + 1) * (B // 2))
                    nc.vector.tensor_scalar_mul(
                        G2[c][:, hb], t_bcast[:, hb], w_pc[:, c : c + 1]
                    )

            # ---- h units (half-B granularity so layer-2 pipelines earlier) ----
            h_tiles = {}
            for n in range(NQ):
                for c in range(C):
                    h_tiles[(n, c)] = ph.tile([128, B], BF, tag="h", name=f"h_{n}_{c}")

            def emit_h_units(half):
                hb = slice(half * (B // 2), (half + 1) * (B // 2))
                for n in range(NQ):
                    for c in range(C):
                        i = n * C + c
                        ht = h_tiles[(n, c)]
                        if i in pe_units:
                            cs = slice(c * 128, (c + 1) * 128)
                            pshs = []
                            for Th in range(2):
                                T = half * 2 + Th
                                bs = slice(T * TS, (T + 1) * TS)
                                psh = pps.tile(
                                    [128, TS], F32, tag="ps", name=f"psh_{n}_{c}_{T}"
                                )
                                nc.tensor.matmul(
                                    psh, identB, g_sb[c][:, bs], start=True, stop=False
                                )
                                pshs.append(psh)
                            for Th in range(2):
                                T = half * 2 + Th
                                bs = slice(T * TS, (T + 1) * TS)
                                nc.tensor.matmul(
                                    pshs[Th], wsc_row[n][:, cs], t_row_bf[:, bs],
                                    start=False, stop=True,
                                )
                            for Th in range(2):
                                T = half * 2 + Th
                                bs = slice(T * TS, (T + 1) * TS)
                                if i in pe_evac_dve:
                                    nc.vector.tensor_scalar_max(ht[:, bs], pshs[Th], 0.0)
                                else:
                                    nc.scalar.activation(ht[:, bs], pshs[Th], AF.Relu)
                        else:
                            # ht = relu(c_n * G2 + g): TS-mul (4x) + TT-add (2x)
                            # + TS-max (4x) all in bf16 fast modes
                            nc.vector.tensor_scalar_mul(ht[:, hb], G2[c][:, hb], CN[n])
                            nc.vector.tensor_tensor(
                                out=ht[:, hb], in0=ht[:, hb], in1=g_sb[c][:, hb],
                                op=OP.add,
                            )
                            if i in relu_act:
                                nc.scalar.activation(ht[:, hb], ht[:, hb], AF.Relu)
                            else:
                                nc.vector.tensor_scalar_max(ht[:, hb], ht[:, hb], 0.0)

            # ---- main pipeline: h units (per half), then per-T layer-2
            #      (col-tiled strips) + softplus + quadrature + final chain ----
            fgroups = [g for g, nodes in enumerate(_node_groups) if nodes]
            psf_count = 0
            for half in range(2):
                emit_h_units(half)
                for Th in range(2):
                    T = half * 2 + Th
                    bs = slice(T * TS, (T + 1) * TS)
                    f_tiles = {}
                    for g, nodes in enumerate(_node_groups):
                        psf = ppsf.tile([128, TS], F32, tag="psf", name=f"psf_{g}_{T}")
                        if psf_count < 2:
                            # first touch of each ring slot: clear garbage rows
                            # so exp of unwritten partitions stays finite
                            nc.vector.memset(psf, 0.0)
                        psf_count += 1
                        for c in range(C):
                            for j, n in enumerate(nodes):
                                nc.tensor.matmul(
                                    psf[32 * j : 32 * j + 8, :],
                                    w2o_sb[c], h_tiles[(n, c)][:, bs],
                                    start=(c == 0), stop=(c == C - 1),
                                    tile_position=(0, 32 * j),
                                )
                            if g == PI_GROUP:
                                nc.tensor.matmul(
                                    psf[32 * PI_STRIP : 32 * PI_STRIP + 8, :],
                                    w2p_sb[c], h1p_sb[c][:, bs],
                                    start=(c == 0), stop=(c == C - 1),
                                    tile_position=(0, 32 * PI_STRIP),
                                )
                        if nodes:
                            top = 32 * (len(nodes) - 1) + 8
                            et = pft.tile([top, TS], BF, tag="et", name=f"et_{g}_{T}")
                            nc.scalar.activation(
                                et, psf[0:top, :], AF.Exp, bias=b2o_col[0:top, :]
                            )
                            ft = pft.tile([top, TS], BF, tag="ft", name=f"ft_{g}_{T}")
                            nc.scalar.activation(ft, et, AF.Ln, bias=1.0)
                            f_tiles[g] = ft
                        if g == PI_GROUP:
                            nc.vector.tensor_scalar_add(
                                lgt_sb[:, bs],
                                psf[32 * PI_STRIP : 32 * PI_STRIP + 8, :],
                                b2p_col[32 * PI_STRIP : 32 * PI_STRIP + 8, :],
                            )

                    # quadrature: pred_sb = -(t/2) sum_n W_n f_n
                    pp = ppred.tile([8, TS], F32, tag="ppred", name=f"ppred_{T}")
                    for k, g in enumerate(fgroups):
                        top = 32 * (len(_node_groups[g]) - 1) + 8
                        nc.tensor.matmul(
                            pp, R_sb[g][0:top, :], f_tiles[g],
                            start=(k == 0), stop=(k == len(fgroups) - 1),
                        )
                    # fold in the per-column t factor during evacuation
                    nc.vector.scalar_tensor_tensor(
                        out=pred_sb[:, bs], in0=pp, scalar=1.0,
                        in1=t_bcast[0:8, bs], op0=OP.mult, op1=OP.mult,
                    )

                    # transpose pred + logits to batch-major
                    pxp = pps.tile([128, 32], F32, tag="ps", name=f"pxp_{T}")
                    pxl = pps.tile([128, 32], F32, tag="ps", name=f"pxl_{T}")
                    for j in range(4):
                        off = T * TS + j * 128
                        nc.tensor.transpose(
                            pxp[:, j * 8 : (j + 1) * 8], pred_sb[:, off : off + 128],
                            ident8,
                        )
                        nc.tensor.transpose(
                            pxl[:, j * 8 : (j + 1) * 8], lgt_sb[:, off : off + 128],
                            ident8,
                        )
                    ts32 = slice(T * 32, (T + 1) * 32)
                    nc.vector.tensor_copy(pred_b[:, ts32], pxp)
                    nc.vector.tensor_copy(logits_b[:, ts32], pxl)

                    # per-T final: softmax(pi), cif, preds
                    nc.scalar.activation(e_b[:, ts32], logits_b[:, ts32], AF.Exp)
                    nc.scalar.activation(eneg[:, ts32], pred_b[:, ts32], AF.Exp)
                    nc.vector.tensor_reduce(
                        sums[:, T * 4 : (T + 1) * 4],
                        e_b[:, ts32].rearrange("p (t k) -> p t k", k=8),
                        axis=AX.X, op=OP.add,
                    )
                    nc.vector.reciprocal(
                        rec[:, T * 4 : (T + 1) * 4], sums[:, T * 4 : (T + 1) * 4]
                    )
                    for jj in range(T * 4, (T + 1) * 4):
                        nc.vector.tensor_scalar_mul(
                            pi_b[:, jj * 8 : (jj + 1) * 8],
                            e_b[:, jj * 8 : (jj + 1) * 8],
                            rec[:, jj : jj + 1],
                        )
                    nc.vector.tensor_scalar(
                        eneg[:, ts32], eneg[:, ts32], -1.0, 1.0, OP.mult, OP.add
                    )
                    nc.vector.tensor_tensor(
                        out=preds_b[:, ts32], in0=eneg[:, ts32], in1=pi_b[:, ts32],
                        op=OP.mult,
                    )

            nc.sync.dma_start(
                out=preds_d.rearrange("(j p) k -> p j k", p=128),
                in_=preds_b.rearrange("p (j k) -> p j k", k=8),
            )
            nc.sync.dma_start(
                out=pi_d.rearrange("(j p) k -> p j k", p=128),
                in_=pi_b.rearrange("p (j k) -> p j k", k=8),
            )

    nc.compile()
    return nc


_NC = None


def _get_nc():
    global _NC
    if _NC is None:
        _NC = build_kernel()
    return _NC


def _shard_inputs(inputs):
    in_maps = []
    for i in range(N_CORES):
        sl = slice(i * B, (i + 1) * B)
        m = {
            "x": np.ascontiguousarray(np.asarray(inputs["x"], np.float32)[sl]),
            "t": np.ascontiguousarray(np.asarray(inputs["t"], np.float32)[sl]),
        }
        for k in ("W1p", "b1p", "W2p", "b2p", "W1o", "b1o", "W2o", "b2o"):
            m[k] = np.asarray(inputs[k], np.float32)
        in_maps.append(m)
    return in_maps


def kernel(**inputs):
    nc = _get_nc()
    in_maps = _shard_inputs(inputs)
    res = run_bass_kernel_spmd(nc, in_maps, core_ids=list(range(N_CORES)))
    preds = np.concatenate([res.results[i]["preds"] for i in range(N_CORES)], axis=0)
    pi = np.concatenate([res.results[i]["pi"] for i in range(N_CORES)], axis=0)
    return (preds, pi)
